# revision 23
# baseline (speedup 1.0000x reference)
"""Trainium2 Bass kernel for nn_DecoderHead (B=4, S=2048, D=1024, H=16).

Sharding: 8 cores = 4 batches x 2 head-groups (8 heads each).
Per core: out[b, :, 512g:512(g+1)] = x[b, :, 512g:512(g+1)] + attn(heads 8g..8g+8).

Device-side design (per core, all fp32 / fp32r matmuls):
  - xT   [1024, 2048]  via PE transposes          (for the q projection)
  - WqT  [1024, 512]   via PE transposes
  - qT   [512, 2048]   = WqT.T @ xT tiles, scaled by 1/sqrt(D)    (PE, fp32r)
  - kT   [dh, sk] per head pair packed [128, 2048] via PE transposes
  - per head pair, per sq-block (512):
      STt = kT_h.T @ qT_h  -> PSUM [sk=128, sq=512] per sk-tile   (2-head
            row-group packed matmuls, K=64 each)
      causal mask: PSUM-accumulate identity.T @ maskneg tile (PE)
      p = exp(STt)  on ScalarE, [128, 1024] spans, PSUM->SBUF
      u'T [65, 512] += [v | 1].T-style matmul: lhsT = v' [sk,65], rhs = p
            (row 64 = softmax denominators, for free)
      u'T -> SBUF, reciprocal on row 64, PE-transpose back to [128, 65],
      o = u * recip (tensor_scalar), residual add (gpsimd), store.
"""

import numpy as np


def _import_concourse():
    try:
        import concourse.bacc  # noqa: F401
    except ImportError:
        import sys

        for p in ("/opt/trn_rl_repo", "/root/.axon_site/_ro/trn_rl_repo"):
            if p not in sys.path:
                sys.path.insert(0, p)
    import concourse.bacc as bacc
    import concourse.tile as tile
    from concourse import bass_utils, mybir

    return bacc, tile, bass_utils, mybir


def build_nc(S=2048, D=1024, HL=8, BLK=512):
    """Build the single-core Bass program (shared SPMD across the 8 cores)."""
    bacc, tile, bass_utils, mybir = _import_concourse()
    from contextlib import ExitStack

    dt = mybir.dt
    F32 = dt.float32
    F32R = dt.float32r
    EXP = mybir.ActivationFunctionType.Exp

    DH = 64
    NST = S // 128  # seq 128-tiles
    ND = D // 128  # d_in 128-tiles
    NB = S // BLK  # sq blocks
    NSUB = BLK // 128  # 128-subtiles per sq block (4)
    NPAIR = HL // 2
    DQ = HL * DH  # local q width (512)
    SCALE = 1.0 / float(np.sqrt(D))

    nc = bacc.Bacc("TRN2", target_bir_lowering=False, debug=False)
    x_d = nc.dram_tensor("x", [S, D], F32, kind="ExternalInput").ap()
    xres_d = nc.dram_tensor("xres", [S, DQ], F32, kind="ExternalInput").ap()
    wq_d = nc.dram_tensor("wq", [DQ, D], F32, kind="ExternalInput").ap()
    k_d = nc.dram_tensor("k", [HL, S, DH], F32, kind="ExternalInput").ap()
    v_d = nc.dram_tensor("v", [HL, S, DH + 1], F32R, kind="ExternalInput").ap()
    id_d = nc.dram_tensor("ident", [128, 128], F32, kind="ExternalInput").ap()
    idr_d = nc.dram_tensor("identr", [128, 128], F32R, kind="ExternalInput").ap()
    mk_d = nc.dram_tensor("maskneg", [NSUB, 128, BLK], F32R, kind="ExternalInput").ap()
    out_d = nc.dram_tensor("out", [S, DQ], F32, kind="ExternalOutput").ap()

    with ExitStack() as ctx:
        tc = ctx.enter_context(tile.TileContext(nc))
        # SBUF pools
        const_p = ctx.enter_context(tc.tile_pool(name="const_p", bufs=1))
        xin_p = ctx.enter_context(tc.tile_pool(name="xin_p", bufs=5))
        xts_p = ctx.enter_context(tc.tile_pool(name="xts_p", bufs=4))
        wq_p = ctx.enter_context(tc.tile_pool(name="wq_p", bufs=2))
        wqt_p = ctx.enter_context(tc.tile_pool(name="wqt_p", bufs=1))
        qt_p = ctx.enter_context(tc.tile_pool(name="qt_p", bufs=2))
        kt_p = ctx.enter_context(tc.tile_pool(name="kt_p", bufs=2))
        kin_p = ctx.enter_context(tc.tile_pool(name="kin_p", bufs=2))
        v_p = ctx.enter_context(tc.tile_pool(name="v_p", bufs=3))
        exp_p = ctx.enter_context(tc.tile_pool(name="exp_p", bufs=3))
        ut_p = ctx.enter_context(tc.tile_pool(name="ut_p", bufs=2))
        o_p = ctx.enter_context(tc.tile_pool(name="o_p", bufs=4))
        xr_p = ctx.enter_context(tc.tile_pool(name="xr_p", bufs=4))
        dram_p = ctx.enter_context(tc.tile_pool(name="dram_p", bufs=1, space="DRAM"))
        # PSUM pools: st 2x[128,1024]=4 banks, u 2x bank = 2, mm 2x bank = 2
        pst = ctx.enter_context(tc.tile_pool(name="pst", bufs=2, space="PSUM"))
        pu = ctx.enter_context(tc.tile_pool(name="pu", bufs=2, space="PSUM"))
        pmm = ctx.enter_context(tc.tile_pool(name="pmm", bufs=2, space="PSUM"))

        # ---- constants ----
        id_sb = const_p.tile([128, 128], F32, name="id_sb")
        nc.sync.dma_start(out=id_sb, in_=id_d)
        id_r = const_p.tile([128, 128], F32R, name="id_r")
        nc.sync.dma_start(out=id_r, in_=idr_d)
        mk_r = const_p.tile([128, NSUB, BLK], F32R, name="mk_r")
        nc.sync.dma_start(out=mk_r, in_=mk_d.rearrange("r p f -> p r f"))

        # ---- k/v loads (pair 0 prefetched before the x/wq phases) ----
        kin_tiles = {}
        v_tiles = {}

        def load_kv(t):
            for h in (2 * t, 2 * t + 1):
                kin = kin_p.tile([128, NST, DH], F32, tag="kin", name=f"kin{h}")
                nc.sync.dma_start(
                    out=kin, in_=k_d[h].rearrange("(t p) d -> p t d", p=128)
                )
                kin_tiles[h] = kin
                vt = v_p.tile([128, NST, DH + 1], F32R, tag="v", name=f"v{h}")
                nc.sync.dma_start(
                    out=vt, in_=v_d[h].rearrange("(t p) d -> p t d", p=128)
                )
                v_tiles[h] = vt

        load_kv(0)

        # ---- Wq load + transpose -> wqT[c] = [128 (d_in), DQ (d_out)] ----
        wqt = []
        for c in range(ND):
            wt = wqt_p.tile([128, DQ], F32R, tag=f"wqt{c}", name=f"wqt{c}")
            wqt.append(wt)
        for m in range(DQ // 128):
            w = wq_p.tile([128, D], F32, tag="wq", name=f"wq_sb{m}")
            nc.sync.dma_start(out=w, in_=wq_d[m * 128 : (m + 1) * 128, :])
            for c in range(ND):
                ps = pmm.tile([128, 128], F32, tag="mm", name=f"wqtp{m}_{c}")
                nc.tensor.matmul(
                    ps, w[:, c * 128 : (c + 1) * 128], id_sb, is_transpose=True
                )
                nc.vector.tensor_copy(
                    out=wqt[c][:, m * 128 : (m + 1) * 128], in_=ps
                )

        # ---- x transpose -> internal DRAM xT[c] = [128 (d_in), S] ----
        xt_d = dram_p.tile([ND, 128, S], F32R, name="xt_d")
        NSG = NST // 4
        for sg in range(NSG):
            xg = []
            for st in range(4 * sg, 4 * sg + 4):
                xi = xin_p.tile([128, D], F32, tag="xin", name=f"xin{st}")
                nc.sync.dma_start(out=xi, in_=x_d[st * 128 : (st + 1) * 128, :])
                xg.append(xi)
            for c in range(ND):
                ps = pmm.tile([128, 512], F32, tag="mm", name=f"xtp{sg}_{c}")
                for il in range(4):
                    nc.tensor.matmul(
                        ps[:, il * 128 : (il + 1) * 128],
                        xg[il][:, c * 128 : (c + 1) * 128],
                        id_sb,
                        is_transpose=True,
                    )
                xw = xts_p.tile([128, 512], F32R, tag="xtw", name=f"xtw{sg}_{c}")
                nc.vector.tensor_copy(out=xw, in_=ps)
                nc.sync.dma_start(
                    out=xt_d[c, :, sg * 512 : (sg + 1) * 512], in_=xw
                )

        # ---- per head-pair ----
        for t in range(NPAIR):
            hA, hB = 2 * t, 2 * t + 1

            # qT tile for this pair: [128 (d_out slice), S].
            # Stream xT c-tiles back from DRAM; contract c outer so each
            # xT tile is loaded once per pair (NB psum banks would not fit,
            # so accumulate per nb in psum with c inner on a block of c).
            qt = qt_p.tile([128, S], F32R, tag="qt", name=f"qt{t}")
            for nb in range(NB):
                qps = pmm.tile([128, BLK], F32, tag="mm", name=f"qps{t}_{nb}")
                for c in range(ND):
                    xc = xts_p.tile(
                        [128, BLK], F32R, tag="xts", name=f"xts{t}_{nb}_{c}"
                    )
                    nc.sync.dma_start(
                        out=xc, in_=xt_d[c, :, nb * BLK : (nb + 1) * BLK]
                    )
                    nc.tensor.matmul(
                        qps,
                        wqt[c][:, t * 128 : (t + 1) * 128],
                        xc,
                        start=(c == 0),
                        stop=(c == ND - 1),
                    )
                nc.vector.tensor_scalar_mul(
                    qt[:, nb * BLK : (nb + 1) * BLK], qps, SCALE
                )

            # kT pair tile [128, S]: rows 0:64 head A, 64:128 head B.
            # PSUM outputs must start at partition 0, so head B goes through
            # an SBUF staging tile and an SBUF->SBUF DMA partition shift.
            kt = kt_p.tile([128, S], F32R, tag="kt", name=f"kt{t}")
            kstg = kt_p.tile([64, S], F32R, tag="kstg", bufs=1, name=f"kstg{t}")
            for i0 in range(0, NST, 4):
                psA = pmm.tile([64, 512], F32, tag="mm", name=f"ktpa{t}_{i0}")
                psB = pmm.tile([64, 512], F32, tag="mm", name=f"ktpb{t}_{i0}")
                for il in range(4):
                    i = i0 + il
                    nc.tensor.matmul(
                        psA[:, il * 128 : (il + 1) * 128],
                        kin_tiles[hA][:, i, :],
                        id_sb,
                        is_transpose=True,
                    )
                    nc.tensor.matmul(
                        psB[:, il * 128 : (il + 1) * 128],
                        kin_tiles[hB][:, i, :],
                        id_sb,
                        is_transpose=True,
                    )
                nc.vector.tensor_copy(
                    out=kt[0:64, i0 * 128 : (i0 + 4) * 128], in_=psA
                )
                nc.vector.tensor_copy(
                    out=kstg[:, i0 * 128 : (i0 + 4) * 128], in_=psB
                )
            nc.sync.dma_start(out=kt[64:128, :], in_=kstg)

            # prefetch next pair's k/v while attention runs
            if t + 1 < NPAIR:
                load_kv(t + 1)

            vA, vB = v_tiles[hA], v_tiles[hB]

            # ---- attention for this pair ----
            for j in range(NB):
                ni = NSUB * (j + 1)  # valid sk tiles for this sq block
                jsl = slice(j * BLK, (j + 1) * BLK)
                uA = pu.tile([DH + 1, BLK], F32, tag="u", name=f"uA{t}_{j}")
                uB = pu.tile([DH + 1, BLK], F32, tag="u", name=f"uB{t}_{j}")
                for ih in range(0, ni, 2):
                    stA = pst.tile([128, 1024], F32, tag="st", name=f"stA{t}{j}{ih}")
                    stB = pst.tile([128, 1024], F32, tag="st", name=f"stB{t}{j}{ih}")
                    for il in range(2):
                        i = ih + il
                        sl = slice(il * BLK, (il + 1) * BLK)
                        isl = slice(i * 128, (i + 1) * 128)
                        r = i - NSUB * j
                        diag = r >= 0
                        nc.tensor.matmul(
                            stA[:, sl],
                            kt[0:64, isl],
                            qt[0:64, jsl],
                            start=True,
                            stop=not diag,
                            tile_position=(0, 0),
                        )
                        if diag:
                            nc.tensor.matmul(
                                stA[:, sl],
                                id_r,
                                mk_r[:, r, :],
                                start=False,
                                stop=True,
                                tile_position=(0, 0),
                            )
                        nc.tensor.matmul(
                            stB[:, sl],
                            kt[64:128, isl],
                            qt[64:128, jsl],
                            start=True,
                            stop=not diag,
                            tile_position=(64, 0),
                        )
                        if diag:
                            nc.tensor.matmul(
                                stB[:, sl],
                                id_r,
                                mk_r[:, r, :],
                                start=False,
                                stop=True,
                                tile_position=(0, 0),
                            )
                    eA = exp_p.tile([128, 1024], F32R, tag="exp", name=f"eA{t}{j}{ih}")
                    eB = exp_p.tile([128, 1024], F32R, tag="exp", name=f"eB{t}{j}{ih}")
                    nc.scalar.activation(out=eA, in_=stA, func=EXP)
                    nc.scalar.activation(out=eB, in_=stB, func=EXP)
                    for il in range(2):
                        i = ih + il
                        sl = slice(il * BLK, (il + 1) * BLK)
                        nc.tensor.matmul(
                            uA,
                            vA[:, i, :],
                            eA[:, sl],
                            start=(i == 0),
                            stop=(i == ni - 1),
                        )
                        nc.tensor.matmul(
                            uB,
                            vB[:, i, :],
                            eB[:, sl],
                            start=(i == 0),
                            stop=(i == ni - 1),
                        )

                # ---- normalize + output for this (pair, block) ----
                o_tiles = []
                for sub in range(NSUB):
                    ot = o_p.tile([128, 128], F32, tag="o", name=f"o{t}{j}{sub}")
                    o_tiles.append(ot)
                for hofs, uu in ((0, uA), (64, uB)):
                    ut = ut_p.tile([DH + 1, BLK], F32, tag="ut", name=f"ut{t}{j}{hofs}")
                    nc.vector.tensor_copy(out=ut, in_=uu)
                    nc.vector.reciprocal(
                        out=ut[DH : DH + 1, :], in_=ut[DH : DH + 1, :]
                    )
                    for sub in range(NSUB):
                        un = pu.tile([128, DH + 1], F32, tag="u", name=f"un{t}{j}{hofs}{sub}")
                        nc.tensor.matmul(
                            un,
                            ut[:, sub * 128 : (sub + 1) * 128],
                            id_sb[0 : DH + 1, 0 : DH + 1],
                            is_transpose=True,
                        )
                        nc.vector.tensor_scalar_mul(
                            o_tiles[sub][:, hofs : hofs + DH],
                            un[:, 0:DH],
                            un[:, DH : DH + 1],
                        )
                for sub in range(NSUB):
                    rows = slice(j * BLK + sub * 128, j * BLK + (sub + 1) * 128)
                    cols = slice(t * 128, (t + 1) * 128)
                    xr = xr_p.tile([128, 128], F32, tag="xr", name=f"xr{t}{j}{sub}")
                    nc.sync.dma_start(out=xr, in_=xres_d[rows, cols])
                    nc.gpsimd.tensor_add(o_tiles[sub], o_tiles[sub], xr)
                    nc.sync.dma_start(out=out_d[rows, cols], in_=o_tiles[sub])

    nc.compile()
    return nc


def make_masks(BLK=512):
    NSUB = BLK // 128
    r = np.arange(NSUB)[:, None, None]
    p = np.arange(128)[None, :, None]
    f = np.arange(BLK)[None, None, :]
    return np.where(f >= p + 128 * r, 0.0, -30000.0).astype(np.float32)


def round_f32r(a):
    """Round fp32 values to fp32r (clear low 8 mantissa bits, RNE)."""
    u = np.ascontiguousarray(a, dtype=np.float32).view(np.uint32)
    frac = u & np.uint32(0xFF)
    up = (frac > 0x80) | ((frac == 0x80) & ((u >> np.uint32(8)) & np.uint32(1) == 1))
    u2 = (u & np.uint32(0xFFFFFF00)) + np.where(up, np.uint32(0x100), np.uint32(0))
    return u2.view(np.float32).reshape(a.shape)


def make_in_maps(x, k, v, Wq, HL=8):
    B, S, D = x.shape
    H = k.shape[1]
    G = H // HL
    DQ = HL * 64
    ident = np.eye(128, dtype=np.float32)
    maskneg = make_masks()
    vones = np.concatenate(
        [round_f32r(v), np.ones(v.shape[:-1] + (1,), dtype=np.float32)], axis=-1
    )
    in_maps = []
    for core in range(B * G):
        b, g = divmod(core, G)
        in_maps.append(
            {
                "x": np.ascontiguousarray(x[b]),
                "xres": np.ascontiguousarray(x[b][:, g * DQ : (g + 1) * DQ]),
                "wq": np.ascontiguousarray(Wq[g * DQ : (g + 1) * DQ]),
                "k": np.ascontiguousarray(k[b, g * HL : (g + 1) * HL]),
                "v": vones[b, g * HL : (g + 1) * HL],
                "ident": ident,
                "identr": ident,
                "maskneg": maskneg,
            }
        )
    return in_maps


_NC_CACHE = {}


def _ensure_ntff_hook():
    """Provide antenv.axon_hooks if the image lacks it (trace=True path)."""
    import sys

    try:
        from antenv.axon_hooks import get_axon_ntff_profile_hook  # noqa: F401

        return
    except ImportError:
        pass
    import contextlib
    import ctypes
    import types

    so_path = "/opt/axon/libaxon_pjrt.so"
    hook = None
    try:
        lib = ctypes.CDLL(so_path)
        if hasattr(lib, "axon_start_nrt_profile"):
            lib.axon_start_nrt_profile.argtypes = [
                ctypes.POINTER(ctypes.c_int64),
                ctypes.c_size_t,
            ]
            lib.axon_start_nrt_profile.restype = ctypes.c_int64
            lib.axon_stop_nrt_profile.argtypes = [ctypes.c_char_p]
            lib.axon_stop_nrt_profile.restype = ctypes.c_int64

            @contextlib.contextmanager
            def _hook(output_dir, device_ids):
                import jax

                jax.devices()
                if device_ids:
                    ids = (ctypes.c_int64 * len(device_ids))(*device_ids)
                    rc = lib.axon_start_nrt_profile(ids, len(device_ids))
                else:
                    rc = lib.axon_start_nrt_profile(None, 0)
                if rc != 0:
                    raise RuntimeError(f"axon_start_nrt_profile rc={rc}")
                try:
                    yield
                finally:
                    n = lib.axon_stop_nrt_profile(str(output_dir).encode())
                    print(f"profile: {n} file(s) written to {output_dir}")

            hook = _hook
    except OSError:
        pass

    mod = types.ModuleType("antenv.axon_hooks")
    mod.get_axon_ntff_profile_hook = lambda: hook
    mod.set_axon_ntff_profile_hook = lambda h: None
    sys.modules["antenv.axon_hooks"] = mod


def run(x, k, v, Wq, trace=False, **kw):
    bacc, tile, bass_utils, mybir = _import_concourse()
    B, S, D = x.shape
    H = k.shape[1]
    HL = 8
    G = H // HL
    DQ = HL * 64
    if trace:
        _ensure_ntff_hook()
    key = (S, D, HL)
    if key not in _NC_CACHE:
        _NC_CACHE[key] = build_nc(S=S, D=D, HL=HL)
    nc = _NC_CACHE[key]
    in_maps = make_in_maps(x, k, v, Wq, HL=HL)
    res = bass_utils.run_bass_kernel_spmd(
        nc, in_maps, core_ids=list(range(B * G)), trace=trace, **kw
    )
    out = np.empty((B, S, D), dtype=x.dtype)
    for core in range(B * G):
        b, g = divmod(core, G)
        out[b][:, g * DQ : (g + 1) * DQ] = res.results[core]["out"]
    return out, res


def kernel(**inputs):
    out, _ = run(inputs["x"], inputs["k"], inputs["v"], inputs["Wq"])
    return out


# revision 26
# speedup vs baseline: 1.0674x; 1.0674x over previous
"""Trainium2 Bass kernel for nn_DecoderHead (B=4, S=2048, D=1024, H=16).

Sharding: 8 cores = 4 batches x 2 head-groups (8 heads each).
Per core: out[b, :, 512g:512(g+1)] = x[b, :, 512g:512(g+1)] + attn(heads 8g..8g+8).

Device-side design (per core, all fp32 / fp32r matmuls):
  - xT   [1024, 2048]  via PE transposes          (for the q projection)
  - WqT  [1024, 512]   via PE transposes
  - qT   [512, 2048]   = WqT.T @ xT tiles, scaled by 1/sqrt(D)    (PE, fp32r)
  - kT   [dh, sk] per head pair packed [128, 2048] via PE transposes
  - per head pair, per sq-block (512):
      STt = kT_h.T @ qT_h  -> PSUM [sk=128, sq=512] per sk-tile   (2-head
            row-group packed matmuls, K=64 each)
      causal mask: PSUM-accumulate identity.T @ maskneg tile (PE)
      p = exp(STt)  on ScalarE, [128, 1024] spans, PSUM->SBUF
      u'T [65, 512] += [v | 1].T-style matmul: lhsT = v' [sk,65], rhs = p
            (row 64 = softmax denominators, for free)
      u'T -> SBUF, reciprocal on row 64, PE-transpose back to [128, 65],
      o = u * recip (tensor_scalar), residual add (gpsimd), store.
"""

import numpy as np


def _import_concourse():
    try:
        import concourse.bacc  # noqa: F401
    except ImportError:
        import sys

        for p in ("/opt/trn_rl_repo", "/root/.axon_site/_ro/trn_rl_repo"):
            if p not in sys.path:
                sys.path.insert(0, p)
    import concourse.bacc as bacc
    import concourse.tile as tile
    from concourse import bass_utils, mybir

    return bacc, tile, bass_utils, mybir


def build_nc(S=2048, D=1024, HL=8, BLK=512):
    """Build the single-core Bass program (shared SPMD across the 8 cores)."""
    bacc, tile, bass_utils, mybir = _import_concourse()
    from contextlib import ExitStack

    dt = mybir.dt
    F32 = dt.float32
    F32R = dt.float32r
    BF16 = dt.bfloat16
    EXP = mybir.ActivationFunctionType.Exp

    DH = 64
    NST = S // 128  # seq 128-tiles
    ND = D // 128  # d_in 128-tiles
    NB = S // BLK  # sq blocks
    NSUB = BLK // 128  # 128-subtiles per sq block (4)
    NPAIR = HL // 2
    DQ = HL * DH  # local q width (512)
    SCALE = 1.0 / float(np.sqrt(D))

    nc = bacc.Bacc("TRN2", target_bir_lowering=False, debug=False)
    x_d = nc.dram_tensor("x", [S, D], F32, kind="ExternalInput").ap()
    xres_d = nc.dram_tensor("xres", [S, DQ], F32, kind="ExternalInput").ap()
    wq_d = nc.dram_tensor("wq", [DQ, D], F32, kind="ExternalInput").ap()
    k_d = nc.dram_tensor("k", [HL, S, DH], F32, kind="ExternalInput").ap()
    v_d = nc.dram_tensor("v", [HL, S, DH + 1], F32R, kind="ExternalInput").ap()
    id_d = nc.dram_tensor("ident", [128, 128], F32, kind="ExternalInput").ap()
    idr_d = nc.dram_tensor("identr", [128, 128], BF16, kind="ExternalInput").ap()
    mk_d = nc.dram_tensor("maskneg", [NSUB, 128, BLK], BF16, kind="ExternalInput").ap()
    out_d = nc.dram_tensor("out", [S, DQ], F32, kind="ExternalOutput").ap()

    with ExitStack() as ctx:
        tc = ctx.enter_context(tile.TileContext(nc))
        # SBUF pools
        const_p = ctx.enter_context(tc.tile_pool(name="const_p", bufs=1))
        xin_p = ctx.enter_context(tc.tile_pool(name="xin_p", bufs=5))
        xts_p = ctx.enter_context(tc.tile_pool(name="xts_p", bufs=4))
        wq_p = ctx.enter_context(tc.tile_pool(name="wq_p", bufs=2))
        wqt_p = ctx.enter_context(tc.tile_pool(name="wqt_p", bufs=1))
        qt_p = ctx.enter_context(tc.tile_pool(name="qt_p", bufs=2))
        kt_p = ctx.enter_context(tc.tile_pool(name="kt_p", bufs=2))
        kin_p = ctx.enter_context(tc.tile_pool(name="kin_p", bufs=2))
        v_p = ctx.enter_context(tc.tile_pool(name="v_p", bufs=3))
        exp_p = ctx.enter_context(tc.tile_pool(name="exp_p", bufs=3))
        ut_p = ctx.enter_context(tc.tile_pool(name="ut_p", bufs=2))
        o_p = ctx.enter_context(tc.tile_pool(name="o_p", bufs=4))
        xr_p = ctx.enter_context(tc.tile_pool(name="xr_p", bufs=4))
        dram_p = ctx.enter_context(tc.tile_pool(name="dram_p", bufs=1, space="DRAM"))
        # PSUM pools: st 2x[128,1024]=4 banks, u 2x bank = 2, mm 2x bank = 2
        pst = ctx.enter_context(tc.tile_pool(name="pst", bufs=2, space="PSUM"))
        pu = ctx.enter_context(tc.tile_pool(name="pu", bufs=2, space="PSUM"))
        pmm = ctx.enter_context(tc.tile_pool(name="pmm", bufs=2, space="PSUM"))

        # ---- constants ----
        id_sb = const_p.tile([128, 128], F32, name="id_sb")
        nc.sync.dma_start(out=id_sb, in_=id_d)
        id_r = const_p.tile([128, 128], BF16, name="id_r")
        nc.sync.dma_start(out=id_r, in_=idr_d)
        mk_r = const_p.tile([128, NSUB, BLK], BF16, name="mk_r")
        nc.sync.dma_start(out=mk_r, in_=mk_d.rearrange("r p f -> p r f"))

        # ---- k/v loads (pair 0 prefetched before the x/wq phases) ----
        kin_tiles = {}
        v_tiles = {}

        def load_kv(t):
            for h in (2 * t, 2 * t + 1):
                kin = kin_p.tile([128, NST, DH], F32, tag="kin", name=f"kin{h}")
                nc.gpsimd.dma_start(
                    out=kin, in_=k_d[h].rearrange("(t p) d -> p t d", p=128)
                )
                kin_tiles[h] = kin
                vt = v_p.tile([128, NST, DH + 1], F32R, tag="v", name=f"v{h}")
                nc.gpsimd.dma_start(
                    out=vt, in_=v_d[h].rearrange("(t p) d -> p t d", p=128)
                )
                v_tiles[h] = vt

        load_kv(0)

        # ---- Wq load + transpose -> wqT[c] = [128 (d_in), DQ (d_out)] ----
        wqt = []
        for c in range(ND):
            wt = wqt_p.tile([128, DQ], F32R, tag=f"wqt{c}", name=f"wqt{c}")
            wqt.append(wt)
        for m in range(DQ // 128):
            w = wq_p.tile([128, D], F32, tag="wq", name=f"wq_sb{m}")
            nc.sync.dma_start(out=w, in_=wq_d[m * 128 : (m + 1) * 128, :])
            for c in range(ND):
                ps = pmm.tile([128, 128], F32, tag="mm", name=f"wqtp{m}_{c}")
                nc.tensor.matmul(
                    ps, w[:, c * 128 : (c + 1) * 128], id_sb, is_transpose=True
                )
                nc.vector.tensor_copy(
                    out=wqt[c][:, m * 128 : (m + 1) * 128], in_=ps
                )

        # ---- x transpose -> internal DRAM xT[c] = [128 (d_in), S] ----
        xt_d = dram_p.tile([ND, 128, S], F32R, name="xt_d")
        NSG = NST // 4
        for sg in range(NSG):
            xg = []
            for st in range(4 * sg, 4 * sg + 4):
                xi = xin_p.tile([128, D], F32, tag="xin", name=f"xin{st}")
                nc.gpsimd.dma_start(out=xi, in_=x_d[st * 128 : (st + 1) * 128, :])
                xg.append(xi)
            for c in range(ND):
                ps = pmm.tile([128, 512], F32, tag="mm", name=f"xtp{sg}_{c}")
                for il in range(4):
                    nc.tensor.matmul(
                        ps[:, il * 128 : (il + 1) * 128],
                        xg[il][:, c * 128 : (c + 1) * 128],
                        id_sb,
                        is_transpose=True,
                    )
                xw = xts_p.tile([128, 512], F32R, tag="xtw", name=f"xtw{sg}_{c}")
                nc.vector.tensor_copy(out=xw, in_=ps)
                nc.scalar.dma_start(
                    out=xt_d[c, :, sg * 512 : (sg + 1) * 512], in_=xw
                )

        # ---- per head-pair ----
        for t in range(NPAIR):
            hA, hB = 2 * t, 2 * t + 1

            # qT tile for this pair: [128 (d_out slice), S].
            # Stream xT c-tiles back from DRAM; contract c outer so each
            # xT tile is loaded once per pair (NB psum banks would not fit,
            # so accumulate per nb in psum with c inner on a block of c).
            qt = qt_p.tile([128, S], F32R, tag="qt", name=f"qt{t}")
            for nbb in range(NB // 2):
                qpa = pmm.tile([128, BLK], F32, tag="mm", name=f"qpa{t}_{nbb}")
                qpb = pmm.tile([128, BLK], F32, tag="mm", name=f"qpb{t}_{nbb}")
                for c in range(ND):
                    xc = xts_p.tile(
                        [128, 2 * BLK], F32R, tag="xts", name=f"xts{t}_{nbb}_{c}"
                    )
                    nc.sync.dma_start(
                        out=xc,
                        in_=xt_d[c, :, nbb * 2 * BLK : (nbb + 1) * 2 * BLK],
                    )
                    wslice = wqt[c][:, t * 128 : (t + 1) * 128]
                    nc.tensor.matmul(
                        qpa, wslice, xc[:, 0:BLK],
                        start=(c == 0), stop=(c == ND - 1),
                    )
                    nc.tensor.matmul(
                        qpb, wslice, xc[:, BLK : 2 * BLK],
                        start=(c == 0), stop=(c == ND - 1),
                    )
                nc.vector.tensor_scalar_mul(
                    qt[:, (2 * nbb) * BLK : (2 * nbb + 1) * BLK], qpa, SCALE
                )
                nc.vector.tensor_scalar_mul(
                    qt[:, (2 * nbb + 1) * BLK : (2 * nbb + 2) * BLK], qpb, SCALE
                )

            # kT pair tile [128, S]: rows 0:64 head A, 64:128 head B.
            # PSUM outputs must start at partition 0, so head B goes through
            # an SBUF staging tile and an SBUF->SBUF DMA partition shift.
            kt = kt_p.tile([128, S], F32R, tag="kt", name=f"kt{t}")
            kstg = kt_p.tile([64, S], F32R, tag="kstg", bufs=1, name=f"kstg{t}")
            for i0 in range(0, NST, 4):
                psA = pmm.tile([64, 512], F32, tag="mm", name=f"ktpa{t}_{i0}")
                psB = pmm.tile([64, 512], F32, tag="mm", name=f"ktpb{t}_{i0}")
                for il in range(4):
                    i = i0 + il
                    nc.tensor.matmul(
                        psA[:, il * 128 : (il + 1) * 128],
                        kin_tiles[hA][:, i, :],
                        id_sb,
                        is_transpose=True,
                    )
                    nc.tensor.matmul(
                        psB[:, il * 128 : (il + 1) * 128],
                        kin_tiles[hB][:, i, :],
                        id_sb,
                        is_transpose=True,
                    )
                nc.vector.tensor_copy(
                    out=kt[0:64, i0 * 128 : (i0 + 4) * 128], in_=psA
                )
                nc.vector.tensor_copy(
                    out=kstg[:, i0 * 128 : (i0 + 4) * 128], in_=psB
                )
            nc.gpsimd.dma_start(out=kt[64:128, :], in_=kstg)

            # prefetch next pair's k/v while attention runs
            if t + 1 < NPAIR:
                load_kv(t + 1)

            vA, vB = v_tiles[hA], v_tiles[hB]

            # ---- attention for this pair ----
            for j in range(NB):
                ni = NSUB * (j + 1)  # valid sk tiles for this sq block
                jsl = slice(j * BLK, (j + 1) * BLK)
                uA = pu.tile([DH + 1, BLK], F32, tag="u", name=f"uA{t}_{j}")
                uB = pu.tile([DH + 1, BLK], F32, tag="u", name=f"uB{t}_{j}")
                for ih in range(0, ni, 2):
                    stA = pst.tile([128, 1024], F32, tag="st", name=f"stA{t}{j}{ih}")
                    stB = pst.tile([128, 1024], F32, tag="st", name=f"stB{t}{j}{ih}")
                    for il in range(2):
                        i = ih + il
                        sl = slice(il * BLK, (il + 1) * BLK)
                        isl = slice(i * 128, (i + 1) * 128)
                        r = i - NSUB * j
                        diag = r >= 0
                        nc.tensor.matmul(
                            stA[:, sl],
                            kt[0:64, isl],
                            qt[0:64, jsl],
                            start=True,
                            stop=not diag,
                            tile_position=(0, 0),
                        )
                        if diag:
                            nc.tensor.matmul(
                                stA[:, sl],
                                id_r,
                                mk_r[:, r, :],
                                start=False,
                                stop=True,
                                tile_position=(0, 0),
                            )
                        nc.tensor.matmul(
                            stB[:, sl],
                            kt[64:128, isl],
                            qt[64:128, jsl],
                            start=True,
                            stop=not diag,
                            tile_position=(64, 0),
                        )
                        if diag:
                            nc.tensor.matmul(
                                stB[:, sl],
                                id_r,
                                mk_r[:, r, :],
                                start=False,
                                stop=True,
                                tile_position=(0, 0),
                            )
                    eA = exp_p.tile([128, 1024], F32R, tag="exp", name=f"eA{t}{j}{ih}")
                    eB = exp_p.tile([128, 1024], F32R, tag="exp", name=f"eB{t}{j}{ih}")
                    nc.scalar.activation(out=eA, in_=stA, func=EXP)
                    nc.scalar.activation(out=eB, in_=stB, func=EXP)
                    for il in range(2):
                        i = ih + il
                        sl = slice(il * BLK, (il + 1) * BLK)
                        nc.tensor.matmul(
                            uA,
                            vA[:, i, :],
                            eA[:, sl],
                            start=(i == 0),
                            stop=(i == ni - 1),
                        )
                        nc.tensor.matmul(
                            uB,
                            vB[:, i, :],
                            eB[:, sl],
                            start=(i == 0),
                            stop=(i == ni - 1),
                        )

                # ---- normalize + output for this (pair, block) ----
                o4 = o_p.tile([128, NSUB, 128], F32, tag="o", name=f"o{t}{j}")
                xr = xr_p.tile([128, NSUB, 128], F32, tag="xr", name=f"xr{t}{j}")
                cols = slice(t * 128, (t + 1) * 128)
                nc.gpsimd.dma_start(
                    out=xr,
                    in_=xres_d[jsl, cols].rearrange("(s p) c -> p s c", p=128),
                )
                for hofs, uu in ((0, uA), (64, uB)):
                    ut = ut_p.tile([DH + 1, BLK], F32, tag="ut", name=f"ut{t}{j}{hofs}")
                    nc.vector.tensor_copy(out=ut, in_=uu)
                    for sub in range(NSUB):
                        un = pu.tile([128, DH + 1], F32, tag="u", name=f"un{t}{j}{hofs}{sub}")
                        nc.tensor.matmul(
                            un,
                            ut[:, sub * 128 : (sub + 1) * 128],
                            id_sb[0 : DH + 1, 0 : DH + 1],
                            is_transpose=True,
                        )
                        rc = ut_p.tile([128, 1], F32, tag="rc", name=f"rc{t}{j}{hofs}{sub}")
                        nc.vector.reciprocal(out=rc, in_=un[:, DH : DH + 1])
                        nc.vector.tensor_scalar_mul(
                            o4[:, sub, hofs : hofs + DH], un[:, 0:DH], rc
                        )
                nc.gpsimd.tensor_add(o4, o4, xr)
                nc.scalar.dma_start(
                    out=out_d[jsl, cols].rearrange("(s p) c -> p s c", p=128),
                    in_=o4,
                )

    nc.compile()
    return nc


def make_masks(BLK=512):
    NSUB = BLK // 128
    r = np.arange(NSUB)[:, None, None]
    p = np.arange(128)[None, :, None]
    f = np.arange(BLK)[None, None, :]
    return np.where(f >= p + 128 * r, 0.0, -30000.0).astype(np.float32)


def round_f32r(a):
    """Round fp32 values to fp32r (clear low 8 mantissa bits, RNE)."""
    u = np.ascontiguousarray(a, dtype=np.float32).view(np.uint32)
    frac = u & np.uint32(0xFF)
    up = (frac > 0x80) | ((frac == 0x80) & ((u >> np.uint32(8)) & np.uint32(1) == 1))
    u2 = (u & np.uint32(0xFFFFFF00)) + np.where(up, np.uint32(0x100), np.uint32(0))
    return u2.view(np.float32).reshape(a.shape)


def make_in_maps(x, k, v, Wq, HL=8):
    B, S, D = x.shape
    H = k.shape[1]
    G = H // HL
    DQ = HL * 64
    import ml_dtypes

    ident = np.eye(128, dtype=np.float32)
    identb = np.eye(128, dtype=ml_dtypes.bfloat16)
    maskneg = make_masks().astype(ml_dtypes.bfloat16)
    vones = np.concatenate(
        [round_f32r(v), np.ones(v.shape[:-1] + (1,), dtype=np.float32)], axis=-1
    )
    in_maps = []
    for core in range(B * G):
        b, g = divmod(core, G)
        in_maps.append(
            {
                "x": np.ascontiguousarray(x[b]),
                "xres": np.ascontiguousarray(x[b][:, g * DQ : (g + 1) * DQ]),
                "wq": np.ascontiguousarray(Wq[g * DQ : (g + 1) * DQ]),
                "k": np.ascontiguousarray(k[b, g * HL : (g + 1) * HL]),
                "v": vones[b, g * HL : (g + 1) * HL],
                "ident": ident,
                "identr": identb,
                "maskneg": maskneg,
            }
        )
    return in_maps


_NC_CACHE = {}


def _ensure_ntff_hook():
    """Provide antenv.axon_hooks if the image lacks it (trace=True path)."""
    import sys

    try:
        from antenv.axon_hooks import get_axon_ntff_profile_hook  # noqa: F401

        return
    except ImportError:
        pass
    import contextlib
    import ctypes
    import types

    so_path = "/opt/axon/libaxon_pjrt.so"
    hook = None
    try:
        lib = ctypes.CDLL(so_path)
        if hasattr(lib, "axon_start_nrt_profile"):
            lib.axon_start_nrt_profile.argtypes = [
                ctypes.POINTER(ctypes.c_int64),
                ctypes.c_size_t,
            ]
            lib.axon_start_nrt_profile.restype = ctypes.c_int64
            lib.axon_stop_nrt_profile.argtypes = [ctypes.c_char_p]
            lib.axon_stop_nrt_profile.restype = ctypes.c_int64

            @contextlib.contextmanager
            def _hook(output_dir, device_ids):
                import jax

                jax.devices()
                if device_ids:
                    ids = (ctypes.c_int64 * len(device_ids))(*device_ids)
                    rc = lib.axon_start_nrt_profile(ids, len(device_ids))
                else:
                    rc = lib.axon_start_nrt_profile(None, 0)
                if rc != 0:
                    raise RuntimeError(f"axon_start_nrt_profile rc={rc}")
                try:
                    yield
                finally:
                    n = lib.axon_stop_nrt_profile(str(output_dir).encode())
                    print(f"profile: {n} file(s) written to {output_dir}")

            hook = _hook
    except OSError:
        pass

    mod = types.ModuleType("antenv.axon_hooks")
    mod.get_axon_ntff_profile_hook = lambda: hook
    mod.set_axon_ntff_profile_hook = lambda h: None
    sys.modules["antenv.axon_hooks"] = mod


def run(x, k, v, Wq, trace=False, **kw):
    bacc, tile, bass_utils, mybir = _import_concourse()
    B, S, D = x.shape
    H = k.shape[1]
    HL = 8
    G = H // HL
    DQ = HL * 64
    if trace:
        _ensure_ntff_hook()
    key = (S, D, HL)
    if key not in _NC_CACHE:
        _NC_CACHE[key] = build_nc(S=S, D=D, HL=HL)
    nc = _NC_CACHE[key]
    in_maps = make_in_maps(x, k, v, Wq, HL=HL)
    res = bass_utils.run_bass_kernel_spmd(
        nc, in_maps, core_ids=list(range(B * G)), trace=trace, **kw
    )
    out = np.empty((B, S, D), dtype=x.dtype)
    for core in range(B * G):
        b, g = divmod(core, G)
        out[b][:, g * DQ : (g + 1) * DQ] = res.results[core]["out"]
    return out, res


def kernel(**inputs):
    out, _ = run(inputs["x"], inputs["k"], inputs["v"], inputs["Wq"])
    return out


# revision 27
# speedup vs baseline: 1.0740x; 1.0061x over previous
"""Trainium2 Bass kernel for nn_DecoderHead (B=4, S=2048, D=1024, H=16).

Sharding: 8 cores = 4 batches x 2 head-groups (8 heads each).
Per core: out[b, :, 512g:512(g+1)] = x[b, :, 512g:512(g+1)] + attn(heads 8g..8g+8).

Device-side design (per core, all fp32 / fp32r matmuls):
  - xT   [1024, 2048]  via PE transposes          (for the q projection)
  - WqT  [1024, 512]   via PE transposes
  - qT   [512, 2048]   = WqT.T @ xT tiles, scaled by 1/sqrt(D)    (PE, fp32r)
  - kT   [dh, sk] per head pair packed [128, 2048] via PE transposes
  - per head pair, per sq-block (512):
      STt = kT_h.T @ qT_h  -> PSUM [sk=128, sq=512] per sk-tile   (2-head
            row-group packed matmuls, K=64 each)
      causal mask: PSUM-accumulate identity.T @ maskneg tile (PE)
      p = exp(STt)  on ScalarE, [128, 1024] spans, PSUM->SBUF
      u'T [65, 512] += [v | 1].T-style matmul: lhsT = v' [sk,65], rhs = p
            (row 64 = softmax denominators, for free)
      u'T -> SBUF, reciprocal on row 64, PE-transpose back to [128, 65],
      o = u * recip (tensor_scalar), residual add (gpsimd), store.
"""

import numpy as np


def _import_concourse():
    try:
        import concourse.bacc  # noqa: F401
    except ImportError:
        import sys

        for p in ("/opt/trn_rl_repo", "/root/.axon_site/_ro/trn_rl_repo"):
            if p not in sys.path:
                sys.path.insert(0, p)
    import concourse.bacc as bacc
    import concourse.tile as tile
    from concourse import bass_utils, mybir

    return bacc, tile, bass_utils, mybir


def build_nc(S=2048, D=1024, HL=8, BLK=512):
    """Build the single-core Bass program (shared SPMD across the 8 cores)."""
    bacc, tile, bass_utils, mybir = _import_concourse()
    from contextlib import ExitStack

    dt = mybir.dt
    F32 = dt.float32
    F32R = dt.float32r
    BF16 = dt.bfloat16
    EXP = mybir.ActivationFunctionType.Exp

    DH = 64
    NST = S // 128  # seq 128-tiles
    ND = D // 128  # d_in 128-tiles
    NB = S // BLK  # sq blocks
    NSUB = BLK // 128  # 128-subtiles per sq block (4)
    NPAIR = HL // 2
    DQ = HL * DH  # local q width (512)
    SCALE = 1.0 / float(np.sqrt(D))

    nc = bacc.Bacc("TRN2", target_bir_lowering=False, debug=False)
    x_d = nc.dram_tensor("x", [S, D], F32, kind="ExternalInput").ap()
    xres_d = nc.dram_tensor("xres", [S, DQ], F32, kind="ExternalInput").ap()
    wq_d = nc.dram_tensor("wq", [DQ, D], F32, kind="ExternalInput").ap()
    k_d = nc.dram_tensor("k", [HL, S, DH], F32, kind="ExternalInput").ap()
    v_d = nc.dram_tensor("v", [HL, S, DH + 1], F32R, kind="ExternalInput").ap()
    id_d = nc.dram_tensor("ident", [128, 128], F32, kind="ExternalInput").ap()
    idr_d = nc.dram_tensor("identr", [128, 128], BF16, kind="ExternalInput").ap()
    mk_d = nc.dram_tensor("maskneg", [NSUB, 128, BLK], BF16, kind="ExternalInput").ap()
    out_d = nc.dram_tensor("out", [S, DQ], F32, kind="ExternalOutput").ap()

    with ExitStack() as ctx:
        tc = ctx.enter_context(tile.TileContext(nc))
        # SBUF pools
        const_p = ctx.enter_context(tc.tile_pool(name="const_p", bufs=1))
        xin_p = ctx.enter_context(tc.tile_pool(name="xin_p", bufs=5))
        xts_p = ctx.enter_context(tc.tile_pool(name="xts_p", bufs=4))
        wq_p = ctx.enter_context(tc.tile_pool(name="wq_p", bufs=2))
        wqt_p = ctx.enter_context(tc.tile_pool(name="wqt_p", bufs=1))
        qt_p = ctx.enter_context(tc.tile_pool(name="qt_p", bufs=2))
        kt_p = ctx.enter_context(tc.tile_pool(name="kt_p", bufs=2))
        kin_p = ctx.enter_context(tc.tile_pool(name="kin_p", bufs=2))
        v_p = ctx.enter_context(tc.tile_pool(name="v_p", bufs=3))
        exp_p = ctx.enter_context(tc.tile_pool(name="exp_p", bufs=4))
        ut_p = ctx.enter_context(tc.tile_pool(name="ut_p", bufs=4))
        o_p = ctx.enter_context(tc.tile_pool(name="o_p", bufs=4))
        xr_p = ctx.enter_context(tc.tile_pool(name="xr_p", bufs=4))
        dram_p = ctx.enter_context(tc.tile_pool(name="dram_p", bufs=1, space="DRAM"))
        # PSUM pools: st 2x[128,1024]=4 banks, u 2x bank = 2, mm 2x bank = 2
        pst = ctx.enter_context(tc.tile_pool(name="pst", bufs=2, space="PSUM"))
        pu = ctx.enter_context(tc.tile_pool(name="pu", bufs=2, space="PSUM"))
        pmm = ctx.enter_context(tc.tile_pool(name="pmm", bufs=2, space="PSUM"))

        # ---- constants ----
        id_sb = const_p.tile([128, 128], F32, name="id_sb")
        nc.sync.dma_start(out=id_sb, in_=id_d)
        id_r = const_p.tile([128, 128], BF16, name="id_r")
        nc.sync.dma_start(out=id_r, in_=idr_d)
        mk_r = const_p.tile([128, NSUB, BLK], BF16, name="mk_r")
        nc.sync.dma_start(out=mk_r, in_=mk_d.rearrange("r p f -> p r f"))

        # ---- k/v loads (pair 0 prefetched before the x/wq phases) ----
        kin_tiles = {}
        v_tiles = {}

        def load_kv(t):
            for h in (2 * t, 2 * t + 1):
                kin = kin_p.tile([128, NST, DH], F32, tag="kin", name=f"kin{h}")
                nc.gpsimd.dma_start(
                    out=kin, in_=k_d[h].rearrange("(t p) d -> p t d", p=128)
                )
                kin_tiles[h] = kin
                vt = v_p.tile([128, NST, DH + 1], F32R, tag="v", name=f"v{h}")
                nc.gpsimd.dma_start(
                    out=vt, in_=v_d[h].rearrange("(t p) d -> p t d", p=128)
                )
                v_tiles[h] = vt

        load_kv(0)

        # ---- Wq load + transpose -> wqT[c] = [128 (d_in), DQ (d_out)] ----
        wqt = []
        for c in range(ND):
            wt = wqt_p.tile([128, DQ], F32R, tag=f"wqt{c}", name=f"wqt{c}")
            wqt.append(wt)
        for m in range(DQ // 128):
            w = wq_p.tile([128, D], F32, tag="wq", name=f"wq_sb{m}")
            nc.sync.dma_start(out=w, in_=wq_d[m * 128 : (m + 1) * 128, :])
            for c in range(ND):
                ps = pmm.tile([128, 128], F32, tag="mm", name=f"wqtp{m}_{c}")
                nc.tensor.matmul(
                    ps, w[:, c * 128 : (c + 1) * 128], id_sb, is_transpose=True
                )
                nc.vector.tensor_copy(
                    out=wqt[c][:, m * 128 : (m + 1) * 128], in_=ps
                )

        # ---- x transpose -> internal DRAM xT[c] = [128 (d_in), S] ----
        xt_d = dram_p.tile([ND, 128, S], F32R, name="xt_d")
        NSG = NST // 4
        for sg in range(NSG):
            xg = []
            for st in range(4 * sg, 4 * sg + 4):
                xi = xin_p.tile([128, D], F32, tag="xin", name=f"xin{st}")
                nc.gpsimd.dma_start(out=xi, in_=x_d[st * 128 : (st + 1) * 128, :])
                xg.append(xi)
            for c in range(ND):
                ps = pmm.tile([128, 512], F32, tag="mm", name=f"xtp{sg}_{c}")
                for il in range(4):
                    nc.tensor.matmul(
                        ps[:, il * 128 : (il + 1) * 128],
                        xg[il][:, c * 128 : (c + 1) * 128],
                        id_sb,
                        is_transpose=True,
                    )
                xw = xts_p.tile([128, 512], F32R, tag="xtw", name=f"xtw{sg}_{c}")
                nc.vector.tensor_copy(out=xw, in_=ps)
                nc.scalar.dma_start(
                    out=xt_d[c, :, sg * 512 : (sg + 1) * 512], in_=xw
                )

        # ---- per head-pair ----
        for t in range(NPAIR):
            hA, hB = 2 * t, 2 * t + 1

            # kT pair tile [128, S]: rows 0:64 head A, 64:128 head B.
            # PSUM outputs must start at partition 0, so head B goes through
            # an SBUF staging tile and an SBUF->SBUF DMA partition shift.
            kt = kt_p.tile([128, S], F32R, tag="kt", name=f"kt{t}")
            kstg = kt_p.tile([64, S], F32R, tag="kstg", bufs=1, name=f"kstg{t}")
            for i0 in range(0, NST, 4):
                psA = pmm.tile([64, 512], F32, tag="mm", name=f"ktpa{t}_{i0}")
                psB = pmm.tile([64, 512], F32, tag="mm", name=f"ktpb{t}_{i0}")
                for il in range(4):
                    i = i0 + il
                    nc.tensor.matmul(
                        psA[:, il * 128 : (il + 1) * 128],
                        kin_tiles[hA][:, i, :],
                        id_sb,
                        is_transpose=True,
                    )
                    nc.tensor.matmul(
                        psB[:, il * 128 : (il + 1) * 128],
                        kin_tiles[hB][:, i, :],
                        id_sb,
                        is_transpose=True,
                    )
                nc.vector.tensor_copy(
                    out=kt[0:64, i0 * 128 : (i0 + 4) * 128], in_=psA
                )
                nc.vector.tensor_copy(
                    out=kstg[:, i0 * 128 : (i0 + 4) * 128], in_=psB
                )
            nc.gpsimd.dma_start(out=kt[64:128, :], in_=kstg)

            # qT tile for this pair: [128 (d_out slice), S].
            # Stream xT c-tiles back from DRAM; contract c outer so each
            # xT tile is loaded once per pair (NB psum banks would not fit,
            # so accumulate per nb in psum with c inner on a block of c).
            qt = qt_p.tile([128, S], F32R, tag="qt", name=f"qt{t}")
            for nbb in range(NB // 2):
                qpa = pmm.tile([128, BLK], F32, tag="mm", name=f"qpa{t}_{nbb}")
                qpb = pmm.tile([128, BLK], F32, tag="mm", name=f"qpb{t}_{nbb}")
                for c in range(ND):
                    xc = xts_p.tile(
                        [128, 2 * BLK], F32R, tag="xts", name=f"xts{t}_{nbb}_{c}"
                    )
                    nc.sync.dma_start(
                        out=xc,
                        in_=xt_d[c, :, nbb * 2 * BLK : (nbb + 1) * 2 * BLK],
                    )
                    wslice = wqt[c][:, t * 128 : (t + 1) * 128]
                    nc.tensor.matmul(
                        qpa, wslice, xc[:, 0:BLK],
                        start=(c == 0), stop=(c == ND - 1),
                    )
                    nc.tensor.matmul(
                        qpb, wslice, xc[:, BLK : 2 * BLK],
                        start=(c == 0), stop=(c == ND - 1),
                    )
                nc.vector.tensor_scalar_mul(
                    qt[:, (2 * nbb) * BLK : (2 * nbb + 1) * BLK], qpa, SCALE
                )
                nc.vector.tensor_scalar_mul(
                    qt[:, (2 * nbb + 1) * BLK : (2 * nbb + 2) * BLK], qpb, SCALE
                )

            # prefetch next pair's k/v while attention runs
            if t + 1 < NPAIR:
                load_kv(t + 1)

            vA, vB = v_tiles[hA], v_tiles[hB]

            # ---- attention for this pair ----
            for j in range(NB):
                ni = NSUB * (j + 1)  # valid sk tiles for this sq block
                jsl = slice(j * BLK, (j + 1) * BLK)
                uA = pu.tile([DH + 1, BLK], F32, tag="u", name=f"uA{t}_{j}")
                uB = pu.tile([DH + 1, BLK], F32, tag="u", name=f"uB{t}_{j}")
                for ih in range(0, ni, 2):
                    stA = pst.tile([128, 1024], F32, tag="st", name=f"stA{t}{j}{ih}")
                    stB = pst.tile([128, 1024], F32, tag="st", name=f"stB{t}{j}{ih}")
                    for il in range(2):
                        i = ih + il
                        sl = slice(il * BLK, (il + 1) * BLK)
                        isl = slice(i * 128, (i + 1) * 128)
                        r = i - NSUB * j
                        diag = r >= 0
                        nc.tensor.matmul(
                            stA[:, sl],
                            kt[0:64, isl],
                            qt[0:64, jsl],
                            start=True,
                            stop=not diag,
                            tile_position=(0, 0),
                        )
                        if diag:
                            nc.tensor.matmul(
                                stA[:, sl],
                                id_r,
                                mk_r[:, r, :],
                                start=False,
                                stop=True,
                                tile_position=(0, 0),
                            )
                        nc.tensor.matmul(
                            stB[:, sl],
                            kt[64:128, isl],
                            qt[64:128, jsl],
                            start=True,
                            stop=not diag,
                            tile_position=(64, 0),
                        )
                        if diag:
                            nc.tensor.matmul(
                                stB[:, sl],
                                id_r,
                                mk_r[:, r, :],
                                start=False,
                                stop=True,
                                tile_position=(0, 0),
                            )
                    eA = exp_p.tile([128, 1024], F32R, tag="exp", name=f"eA{t}{j}{ih}")
                    eB = exp_p.tile([128, 1024], F32R, tag="exp", name=f"eB{t}{j}{ih}")
                    nc.scalar.activation(out=eA, in_=stA, func=EXP)
                    nc.scalar.activation(out=eB, in_=stB, func=EXP)
                    for il in range(2):
                        i = ih + il
                        sl = slice(il * BLK, (il + 1) * BLK)
                        nc.tensor.matmul(
                            uA,
                            vA[:, i, :],
                            eA[:, sl],
                            start=(i == 0),
                            stop=(i == ni - 1),
                        )
                        nc.tensor.matmul(
                            uB,
                            vB[:, i, :],
                            eB[:, sl],
                            start=(i == 0),
                            stop=(i == ni - 1),
                        )

                # ---- normalize + output for this (pair, block) ----
                o4 = o_p.tile([128, NSUB, 128], F32, tag="o", name=f"o{t}{j}")
                xr = xr_p.tile([128, NSUB, 128], F32, tag="xr", name=f"xr{t}{j}")
                cols = slice(t * 128, (t + 1) * 128)
                nc.gpsimd.dma_start(
                    out=xr,
                    in_=xres_d[jsl, cols].rearrange("(s p) c -> p s c", p=128),
                )
                for hofs, uu in ((0, uA), (64, uB)):
                    ut = ut_p.tile([DH + 1, BLK], F32, tag="ut", name=f"ut{t}{j}{hofs}")
                    nc.vector.tensor_copy(out=ut, in_=uu)
                    for sub in range(NSUB):
                        un = pmm.tile([128, DH + 1], F32, tag="mm", name=f"un{t}{j}{hofs}{sub}")
                        nc.tensor.matmul(
                            un,
                            ut[:, sub * 128 : (sub + 1) * 128],
                            id_sb[0 : DH + 1, 0 : DH + 1],
                            is_transpose=True,
                        )
                        rc = ut_p.tile([128, 1], F32, tag="rc", name=f"rc{t}{j}{hofs}{sub}")
                        nc.vector.reciprocal(out=rc, in_=un[:, DH : DH + 1])
                        nc.vector.tensor_scalar_mul(
                            o4[:, sub, hofs : hofs + DH], un[:, 0:DH], rc
                        )
                nc.gpsimd.tensor_add(o4, o4, xr)
                nc.scalar.dma_start(
                    out=out_d[jsl, cols].rearrange("(s p) c -> p s c", p=128),
                    in_=o4,
                )

    nc.compile()
    return nc


def make_masks(BLK=512):
    NSUB = BLK // 128
    r = np.arange(NSUB)[:, None, None]
    p = np.arange(128)[None, :, None]
    f = np.arange(BLK)[None, None, :]
    return np.where(f >= p + 128 * r, 0.0, -30000.0).astype(np.float32)


def round_f32r(a):
    """Round fp32 values to fp32r (clear low 8 mantissa bits, RNE)."""
    u = np.ascontiguousarray(a, dtype=np.float32).view(np.uint32)
    frac = u & np.uint32(0xFF)
    up = (frac > 0x80) | ((frac == 0x80) & ((u >> np.uint32(8)) & np.uint32(1) == 1))
    u2 = (u & np.uint32(0xFFFFFF00)) + np.where(up, np.uint32(0x100), np.uint32(0))
    return u2.view(np.float32).reshape(a.shape)


def make_in_maps(x, k, v, Wq, HL=8):
    B, S, D = x.shape
    H = k.shape[1]
    G = H // HL
    DQ = HL * 64
    import ml_dtypes

    ident = np.eye(128, dtype=np.float32)
    identb = np.eye(128, dtype=ml_dtypes.bfloat16)
    maskneg = make_masks().astype(ml_dtypes.bfloat16)
    vones = np.concatenate(
        [round_f32r(v), np.ones(v.shape[:-1] + (1,), dtype=np.float32)], axis=-1
    )
    in_maps = []
    for core in range(B * G):
        b, g = divmod(core, G)
        in_maps.append(
            {
                "x": np.ascontiguousarray(x[b]),
                "xres": np.ascontiguousarray(x[b][:, g * DQ : (g + 1) * DQ]),
                "wq": np.ascontiguousarray(Wq[g * DQ : (g + 1) * DQ]),
                "k": np.ascontiguousarray(k[b, g * HL : (g + 1) * HL]),
                "v": vones[b, g * HL : (g + 1) * HL],
                "ident": ident,
                "identr": identb,
                "maskneg": maskneg,
            }
        )
    return in_maps


_NC_CACHE = {}


def _ensure_ntff_hook():
    """Provide antenv.axon_hooks if the image lacks it (trace=True path)."""
    import sys

    try:
        from antenv.axon_hooks import get_axon_ntff_profile_hook  # noqa: F401

        return
    except ImportError:
        pass
    import contextlib
    import ctypes
    import types

    so_path = "/opt/axon/libaxon_pjrt.so"
    hook = None
    try:
        lib = ctypes.CDLL(so_path)
        if hasattr(lib, "axon_start_nrt_profile"):
            lib.axon_start_nrt_profile.argtypes = [
                ctypes.POINTER(ctypes.c_int64),
                ctypes.c_size_t,
            ]
            lib.axon_start_nrt_profile.restype = ctypes.c_int64
            lib.axon_stop_nrt_profile.argtypes = [ctypes.c_char_p]
            lib.axon_stop_nrt_profile.restype = ctypes.c_int64

            @contextlib.contextmanager
            def _hook(output_dir, device_ids):
                import jax

                jax.devices()
                if device_ids:
                    ids = (ctypes.c_int64 * len(device_ids))(*device_ids)
                    rc = lib.axon_start_nrt_profile(ids, len(device_ids))
                else:
                    rc = lib.axon_start_nrt_profile(None, 0)
                if rc != 0:
                    raise RuntimeError(f"axon_start_nrt_profile rc={rc}")
                try:
                    yield
                finally:
                    n = lib.axon_stop_nrt_profile(str(output_dir).encode())
                    print(f"profile: {n} file(s) written to {output_dir}")

            hook = _hook
    except OSError:
        pass

    mod = types.ModuleType("antenv.axon_hooks")
    mod.get_axon_ntff_profile_hook = lambda: hook
    mod.set_axon_ntff_profile_hook = lambda h: None
    sys.modules["antenv.axon_hooks"] = mod


def run(x, k, v, Wq, trace=False, **kw):
    bacc, tile, bass_utils, mybir = _import_concourse()
    B, S, D = x.shape
    H = k.shape[1]
    HL = 8
    G = H // HL
    DQ = HL * 64
    if trace:
        _ensure_ntff_hook()
    key = (S, D, HL)
    if key not in _NC_CACHE:
        _NC_CACHE[key] = build_nc(S=S, D=D, HL=HL)
    nc = _NC_CACHE[key]
    in_maps = make_in_maps(x, k, v, Wq, HL=HL)
    res = bass_utils.run_bass_kernel_spmd(
        nc, in_maps, core_ids=list(range(B * G)), trace=trace, **kw
    )
    out = np.empty((B, S, D), dtype=x.dtype)
    for core in range(B * G):
        b, g = divmod(core, G)
        out[b][:, g * DQ : (g + 1) * DQ] = res.results[core]["out"]
    return out, res


def kernel(**inputs):
    out, _ = run(inputs["x"], inputs["k"], inputs["v"], inputs["Wq"])
    return out


# revision 28
# speedup vs baseline: 1.2830x; 1.1947x over previous
"""Trainium2 Bass kernel for nn_DecoderHead (B=4, S=2048, D=1024, H=16).

Sharding: 8 cores = 4 batches x 2 head-groups (8 heads each).
Per core: out[b, :, 512g:512(g+1)] = x[b, :, 512g:512(g+1)] + attn(heads 8g..8g+8).

Device-side design (per core, all fp32 / fp32r matmuls):
  - xT   [1024, 2048]  via PE transposes          (for the q projection)
  - WqT  [1024, 512]   via PE transposes
  - qT   [512, 2048]   = WqT.T @ xT tiles, scaled by 1/sqrt(D)    (PE, fp32r)
  - kT   [dh, sk] per head pair packed [128, 2048] via PE transposes
  - per head pair, per sq-block (512):
      STt = kT_h.T @ qT_h  -> PSUM [sk=128, sq=512] per sk-tile   (2-head
            row-group packed matmuls, K=64 each)
      causal mask: PSUM-accumulate identity.T @ maskneg tile (PE)
      p = exp(STt)  on ScalarE, [128, 1024] spans, PSUM->SBUF
      u'T [65, 512] += [v | 1].T-style matmul: lhsT = v' [sk,65], rhs = p
            (row 64 = softmax denominators, for free)
      u'T -> SBUF, reciprocal on row 64, PE-transpose back to [128, 65],
      o = u * recip (tensor_scalar), residual add (gpsimd), store.
"""

import numpy as np


def _import_concourse():
    try:
        import concourse.bacc  # noqa: F401
    except ImportError:
        import sys

        for p in ("/opt/trn_rl_repo", "/root/.axon_site/_ro/trn_rl_repo"):
            if p not in sys.path:
                sys.path.insert(0, p)
    import concourse.bacc as bacc
    import concourse.tile as tile
    from concourse import bass_utils, mybir

    return bacc, tile, bass_utils, mybir


def build_nc(S=2048, D=1024, HL=8, BLK=512):
    """Build the single-core Bass program (shared SPMD across the 8 cores)."""
    bacc, tile, bass_utils, mybir = _import_concourse()
    from contextlib import ExitStack

    dt = mybir.dt
    F32 = dt.float32
    F32R = dt.float32r
    BF16 = dt.bfloat16
    EXP = mybir.ActivationFunctionType.Exp

    DH = 64
    NST = S // 128  # seq 128-tiles
    ND = D // 128  # d_in 128-tiles
    NB = S // BLK  # sq blocks
    NSUB = BLK // 128  # 128-subtiles per sq block (4)
    NPAIR = HL // 2
    DQ = HL * DH  # local q width (512)
    SCALE = 1.0 / float(np.sqrt(D))

    nc = bacc.Bacc("TRN2", target_bir_lowering=False, debug=False)
    x_d = nc.dram_tensor("x", [S, D], F32, kind="ExternalInput").ap()
    xres_d = nc.dram_tensor("xres", [S, DQ], F32, kind="ExternalInput").ap()
    wq_d = nc.dram_tensor("wq", [DQ, D], F32, kind="ExternalInput").ap()
    k_d = nc.dram_tensor("k", [HL, S, DH], F32, kind="ExternalInput").ap()
    v_d = nc.dram_tensor("v", [HL, S, DH + 1], F32R, kind="ExternalInput").ap()
    id_d = nc.dram_tensor("ident", [128, 128], F32, kind="ExternalInput").ap()
    idr_d = nc.dram_tensor("identr", [128, 128], BF16, kind="ExternalInput").ap()
    mk_d = nc.dram_tensor("maskneg", [NSUB, 128, BLK], BF16, kind="ExternalInput").ap()
    out_d = nc.dram_tensor("out", [S, DQ], F32, kind="ExternalOutput").ap()

    with ExitStack() as ctx:
        tc = ctx.enter_context(tile.TileContext(nc))
        # SBUF pools
        const_p = ctx.enter_context(tc.tile_pool(name="const_p", bufs=1))
        xin_p = ctx.enter_context(tc.tile_pool(name="xin_p", bufs=3))
        xt_p = ctx.enter_context(tc.tile_pool(name="xt_p", bufs=1))
        wq_p = ctx.enter_context(tc.tile_pool(name="wq_p", bufs=2))
        wqt_p = ctx.enter_context(tc.tile_pool(name="wqt_p", bufs=1))
        qt_p = ctx.enter_context(tc.tile_pool(name="qt_p", bufs=2))
        kt_p = ctx.enter_context(tc.tile_pool(name="kt_p", bufs=2))
        kin_p = ctx.enter_context(tc.tile_pool(name="kin_p", bufs=2))
        v_p = ctx.enter_context(tc.tile_pool(name="v_p", bufs=2))
        exp_p = ctx.enter_context(tc.tile_pool(name="exp_p", bufs=3))
        ut_p = ctx.enter_context(tc.tile_pool(name="ut_p", bufs=3))
        o_p = ctx.enter_context(tc.tile_pool(name="o_p", bufs=2))
        xr_p = ctx.enter_context(tc.tile_pool(name="xr_p", bufs=2))
        # PSUM pools: st 2x[128,1024]=4 banks, u 2x bank = 2, mm 2x bank = 2
        pst = ctx.enter_context(tc.tile_pool(name="pst", bufs=2, space="PSUM"))
        pu = ctx.enter_context(tc.tile_pool(name="pu", bufs=2, space="PSUM"))
        pmm = ctx.enter_context(tc.tile_pool(name="pmm", bufs=2, space="PSUM"))

        # ---- constants ----
        id_sb = const_p.tile([128, 128], F32, name="id_sb")
        nc.sync.dma_start(out=id_sb, in_=id_d)
        id_r = const_p.tile([128, 128], BF16, name="id_r")
        nc.sync.dma_start(out=id_r, in_=idr_d)
        mk_r = const_p.tile([128, NSUB, BLK], BF16, name="mk_r")
        nc.sync.dma_start(out=mk_r, in_=mk_d.rearrange("r p f -> p r f"))

        # ---- k/v loads (pair 0 prefetched before the x/wq phases) ----
        kin_tiles = {}
        v_tiles = {}

        def load_kv(t):
            for h in (2 * t, 2 * t + 1):
                kin = kin_p.tile([128, NST, DH], F32, tag="kin", name=f"kin{h}")
                nc.gpsimd.dma_start(
                    out=kin, in_=k_d[h].rearrange("(t p) d -> p t d", p=128)
                )
                kin_tiles[h] = kin
                vt = v_p.tile([128, NST, DH + 1], F32R, tag="v", name=f"v{h}")
                nc.gpsimd.dma_start(
                    out=vt, in_=v_d[h].rearrange("(t p) d -> p t d", p=128)
                )
                v_tiles[h] = vt

        load_kv(0)

        # ---- Wq load + transpose -> wqT[c] = [128 (d_in), DQ (d_out)] ----
        wqt = []
        for c in range(ND):
            wt = wqt_p.tile([128, DQ], F32R, tag=f"wqt{c}", name=f"wqt{c}")
            wqt.append(wt)
        for m in range(DQ // 128):
            w = wq_p.tile([128, D], F32, tag="wq", name=f"wq_sb{m}")
            nc.sync.dma_start(out=w, in_=wq_d[m * 128 : (m + 1) * 128, :])
            for c in range(ND):
                ps = pmm.tile([128, 128], F32, tag="mm", name=f"wqtp{m}_{c}")
                nc.tensor.matmul(
                    ps, w[:, c * 128 : (c + 1) * 128], id_sb, is_transpose=True
                )
                nc.vector.tensor_copy(
                    out=wqt[c][:, m * 128 : (m + 1) * 128], in_=ps
                )

        # ---- x transpose -> SBUF-resident xT[c] = [128 (d_in), S] ----
        xt = []
        for c in range(ND):
            xtc = xt_p.tile([128, S], F32R, tag=f"xt{c}", name=f"xt{c}")
            xt.append(xtc)
        NSG = NST // 2
        for sg in range(NSG):
            xg = []
            for st in range(2 * sg, 2 * sg + 2):
                xi = xin_p.tile([128, D], F32, tag="xin", name=f"xin{st}")
                nc.gpsimd.dma_start(out=xi, in_=x_d[st * 128 : (st + 1) * 128, :])
                xg.append(xi)
            for c in range(ND):
                ps = pmm.tile([128, 256], F32, tag="mm", name=f"xtp{sg}_{c}")
                for il in range(2):
                    nc.tensor.matmul(
                        ps[:, il * 128 : (il + 1) * 128],
                        xg[il][:, c * 128 : (c + 1) * 128],
                        id_sb,
                        is_transpose=True,
                    )
                nc.vector.tensor_copy(
                    out=xt[c][:, sg * 256 : (sg + 1) * 256], in_=ps
                )

        # ---- per head-pair ----
        for t in range(NPAIR):
            hA, hB = 2 * t, 2 * t + 1

            # kT pair tile [128, S]: rows 0:64 head A, 64:128 head B.
            # PSUM outputs must start at partition 0, so head B goes through
            # an SBUF staging tile and an SBUF->SBUF DMA partition shift.
            kt = kt_p.tile([128, S], F32R, tag="kt", name=f"kt{t}")
            kstg = kt_p.tile([64, S], F32R, tag="kstg", bufs=1, name=f"kstg{t}")
            for i0 in range(0, NST, 4):
                psA = pmm.tile([64, 512], F32, tag="mm", name=f"ktpa{t}_{i0}")
                psB = pmm.tile([64, 512], F32, tag="mm", name=f"ktpb{t}_{i0}")
                for il in range(4):
                    i = i0 + il
                    nc.tensor.matmul(
                        psA[:, il * 128 : (il + 1) * 128],
                        kin_tiles[hA][:, i, :],
                        id_sb,
                        is_transpose=True,
                    )
                    nc.tensor.matmul(
                        psB[:, il * 128 : (il + 1) * 128],
                        kin_tiles[hB][:, i, :],
                        id_sb,
                        is_transpose=True,
                    )
                nc.vector.tensor_copy(
                    out=kt[0:64, i0 * 128 : (i0 + 4) * 128], in_=psA
                )
                nc.vector.tensor_copy(
                    out=kstg[:, i0 * 128 : (i0 + 4) * 128], in_=psB
                )
            nc.gpsimd.dma_start(out=kt[64:128, :], in_=kstg)

            # qT tile for this pair: [128 (d_out slice), S].
            # Stream xT c-tiles back from DRAM; contract c outer so each
            # xT tile is loaded once per pair (NB psum banks would not fit,
            # so accumulate per nb in psum with c inner on a block of c).
            qt = qt_p.tile([128, S], F32R, tag="qt", name=f"qt{t}")
            for nbb in range(NB // 2):
                qpa = pmm.tile([128, BLK], F32, tag="mm", name=f"qpa{t}_{nbb}")
                qpb = pmm.tile([128, BLK], F32, tag="mm", name=f"qpb{t}_{nbb}")
                for c in range(ND):
                    wslice = wqt[c][:, t * 128 : (t + 1) * 128]
                    nc.tensor.matmul(
                        qpa, wslice,
                        xt[c][:, (2 * nbb) * BLK : (2 * nbb + 1) * BLK],
                        start=(c == 0), stop=(c == ND - 1),
                    )
                    nc.tensor.matmul(
                        qpb, wslice,
                        xt[c][:, (2 * nbb + 1) * BLK : (2 * nbb + 2) * BLK],
                        start=(c == 0), stop=(c == ND - 1),
                    )
                nc.vector.tensor_scalar_mul(
                    qt[:, (2 * nbb) * BLK : (2 * nbb + 1) * BLK], qpa, SCALE
                )
                nc.vector.tensor_scalar_mul(
                    qt[:, (2 * nbb + 1) * BLK : (2 * nbb + 2) * BLK], qpb, SCALE
                )

            # prefetch next pair's k/v while attention runs
            if t + 1 < NPAIR:
                load_kv(t + 1)

            vA, vB = v_tiles[hA], v_tiles[hB]

            # ---- attention for this pair ----
            for j in range(NB):
                ni = NSUB * (j + 1)  # valid sk tiles for this sq block
                jsl = slice(j * BLK, (j + 1) * BLK)
                uA = pu.tile([DH + 1, BLK], F32, tag="u", name=f"uA{t}_{j}")
                uB = pu.tile([DH + 1, BLK], F32, tag="u", name=f"uB{t}_{j}")
                for ih in range(0, ni, 2):
                    stA = pst.tile([128, 1024], F32, tag="st", name=f"stA{t}{j}{ih}")
                    stB = pst.tile([128, 1024], F32, tag="st", name=f"stB{t}{j}{ih}")
                    for il in range(2):
                        i = ih + il
                        sl = slice(il * BLK, (il + 1) * BLK)
                        isl = slice(i * 128, (i + 1) * 128)
                        r = i - NSUB * j
                        diag = r >= 0
                        nc.tensor.matmul(
                            stA[:, sl],
                            kt[0:64, isl],
                            qt[0:64, jsl],
                            start=True,
                            stop=not diag,
                            tile_position=(0, 0),
                        )
                        if diag:
                            nc.tensor.matmul(
                                stA[:, sl],
                                id_r,
                                mk_r[:, r, :],
                                start=False,
                                stop=True,
                                tile_position=(0, 0),
                            )
                        nc.tensor.matmul(
                            stB[:, sl],
                            kt[64:128, isl],
                            qt[64:128, jsl],
                            start=True,
                            stop=not diag,
                            tile_position=(64, 0),
                        )
                        if diag:
                            nc.tensor.matmul(
                                stB[:, sl],
                                id_r,
                                mk_r[:, r, :],
                                start=False,
                                stop=True,
                                tile_position=(0, 0),
                            )
                    eA = exp_p.tile([128, 1024], F32R, tag="exp", name=f"eA{t}{j}{ih}")
                    eB = exp_p.tile([128, 1024], F32R, tag="exp", name=f"eB{t}{j}{ih}")
                    nc.scalar.activation(out=eA, in_=stA, func=EXP)
                    nc.scalar.activation(out=eB, in_=stB, func=EXP)
                    for il in range(2):
                        i = ih + il
                        sl = slice(il * BLK, (il + 1) * BLK)
                        nc.tensor.matmul(
                            uA,
                            vA[:, i, :],
                            eA[:, sl],
                            start=(i == 0),
                            stop=(i == ni - 1),
                        )
                        nc.tensor.matmul(
                            uB,
                            vB[:, i, :],
                            eB[:, sl],
                            start=(i == 0),
                            stop=(i == ni - 1),
                        )

                # ---- normalize + output for this (pair, block) ----
                o4 = o_p.tile([128, NSUB, 128], F32, tag="o", name=f"o{t}{j}")
                xr = xr_p.tile([128, NSUB, 128], F32, tag="xr", name=f"xr{t}{j}")
                cols = slice(t * 128, (t + 1) * 128)
                nc.gpsimd.dma_start(
                    out=xr,
                    in_=xres_d[jsl, cols].rearrange("(s p) c -> p s c", p=128),
                )
                for hofs, uu in ((0, uA), (64, uB)):
                    ut = ut_p.tile([DH + 1, BLK], F32, tag="ut", name=f"ut{t}{j}{hofs}")
                    nc.vector.tensor_copy(out=ut, in_=uu)
                    for sub in range(NSUB):
                        un = pmm.tile([128, DH + 1], F32, tag="mm", name=f"un{t}{j}{hofs}{sub}")
                        nc.tensor.matmul(
                            un,
                            ut[:, sub * 128 : (sub + 1) * 128],
                            id_sb[0 : DH + 1, 0 : DH + 1],
                            is_transpose=True,
                        )
                        rc = ut_p.tile([128, 1], F32, tag="rc", name=f"rc{t}{j}{hofs}{sub}")
                        nc.vector.reciprocal(out=rc, in_=un[:, DH : DH + 1])
                        nc.vector.tensor_scalar_mul(
                            o4[:, sub, hofs : hofs + DH], un[:, 0:DH], rc
                        )
                nc.gpsimd.tensor_add(o4, o4, xr)
                nc.scalar.dma_start(
                    out=out_d[jsl, cols].rearrange("(s p) c -> p s c", p=128),
                    in_=o4,
                )

    nc.compile()
    return nc


def make_masks(BLK=512):
    NSUB = BLK // 128
    r = np.arange(NSUB)[:, None, None]
    p = np.arange(128)[None, :, None]
    f = np.arange(BLK)[None, None, :]
    return np.where(f >= p + 128 * r, 0.0, -30000.0).astype(np.float32)


def round_f32r(a):
    """Round fp32 values to fp32r (clear low 8 mantissa bits, RNE)."""
    u = np.ascontiguousarray(a, dtype=np.float32).view(np.uint32)
    frac = u & np.uint32(0xFF)
    up = (frac > 0x80) | ((frac == 0x80) & ((u >> np.uint32(8)) & np.uint32(1) == 1))
    u2 = (u & np.uint32(0xFFFFFF00)) + np.where(up, np.uint32(0x100), np.uint32(0))
    return u2.view(np.float32).reshape(a.shape)


def make_in_maps(x, k, v, Wq, HL=8):
    B, S, D = x.shape
    H = k.shape[1]
    G = H // HL
    DQ = HL * 64
    import ml_dtypes

    ident = np.eye(128, dtype=np.float32)
    identb = np.eye(128, dtype=ml_dtypes.bfloat16)
    maskneg = make_masks().astype(ml_dtypes.bfloat16)
    vones = np.concatenate(
        [round_f32r(v), np.ones(v.shape[:-1] + (1,), dtype=np.float32)], axis=-1
    )
    in_maps = []
    for core in range(B * G):
        b, g = divmod(core, G)
        in_maps.append(
            {
                "x": np.ascontiguousarray(x[b]),
                "xres": np.ascontiguousarray(x[b][:, g * DQ : (g + 1) * DQ]),
                "wq": np.ascontiguousarray(Wq[g * DQ : (g + 1) * DQ]),
                "k": np.ascontiguousarray(k[b, g * HL : (g + 1) * HL]),
                "v": vones[b, g * HL : (g + 1) * HL],
                "ident": ident,
                "identr": identb,
                "maskneg": maskneg,
            }
        )
    return in_maps


_NC_CACHE = {}


def _ensure_ntff_hook():
    """Provide antenv.axon_hooks if the image lacks it (trace=True path)."""
    import sys

    try:
        from antenv.axon_hooks import get_axon_ntff_profile_hook  # noqa: F401

        return
    except ImportError:
        pass
    import contextlib
    import ctypes
    import types

    so_path = "/opt/axon/libaxon_pjrt.so"
    hook = None
    try:
        lib = ctypes.CDLL(so_path)
        if hasattr(lib, "axon_start_nrt_profile"):
            lib.axon_start_nrt_profile.argtypes = [
                ctypes.POINTER(ctypes.c_int64),
                ctypes.c_size_t,
            ]
            lib.axon_start_nrt_profile.restype = ctypes.c_int64
            lib.axon_stop_nrt_profile.argtypes = [ctypes.c_char_p]
            lib.axon_stop_nrt_profile.restype = ctypes.c_int64

            @contextlib.contextmanager
            def _hook(output_dir, device_ids):
                import jax

                jax.devices()
                if device_ids:
                    ids = (ctypes.c_int64 * len(device_ids))(*device_ids)
                    rc = lib.axon_start_nrt_profile(ids, len(device_ids))
                else:
                    rc = lib.axon_start_nrt_profile(None, 0)
                if rc != 0:
                    raise RuntimeError(f"axon_start_nrt_profile rc={rc}")
                try:
                    yield
                finally:
                    n = lib.axon_stop_nrt_profile(str(output_dir).encode())
                    print(f"profile: {n} file(s) written to {output_dir}")

            hook = _hook
    except OSError:
        pass

    mod = types.ModuleType("antenv.axon_hooks")
    mod.get_axon_ntff_profile_hook = lambda: hook
    mod.set_axon_ntff_profile_hook = lambda h: None
    sys.modules["antenv.axon_hooks"] = mod


def run(x, k, v, Wq, trace=False, **kw):
    bacc, tile, bass_utils, mybir = _import_concourse()
    B, S, D = x.shape
    H = k.shape[1]
    HL = 8
    G = H // HL
    DQ = HL * 64
    if trace:
        _ensure_ntff_hook()
    key = (S, D, HL)
    if key not in _NC_CACHE:
        _NC_CACHE[key] = build_nc(S=S, D=D, HL=HL)
    nc = _NC_CACHE[key]
    in_maps = make_in_maps(x, k, v, Wq, HL=HL)
    res = bass_utils.run_bass_kernel_spmd(
        nc, in_maps, core_ids=list(range(B * G)), trace=trace, **kw
    )
    out = np.empty((B, S, D), dtype=x.dtype)
    for core in range(B * G):
        b, g = divmod(core, G)
        out[b][:, g * DQ : (g + 1) * DQ] = res.results[core]["out"]
    return out, res


def kernel(**inputs):
    out, _ = run(inputs["x"], inputs["k"], inputs["v"], inputs["Wq"])
    return out


# revision 29
# speedup vs baseline: 1.3446x; 1.0480x over previous
"""Trainium2 Bass kernel for nn_DecoderHead (B=4, S=2048, D=1024, H=16).

Sharding: 8 cores = 4 batches x 2 head-groups (8 heads each).
Per core: out[b, :, 512g:512(g+1)] = x[b, :, 512g:512(g+1)] + attn(heads 8g..8g+8).

Device-side design (per core, all fp32 / fp32r matmuls):
  - xT   [1024, 2048]  via PE transposes          (for the q projection)
  - WqT  [1024, 512]   via PE transposes
  - qT   [512, 2048]   = WqT.T @ xT tiles, scaled by 1/sqrt(D)    (PE, fp32r)
  - kT   [dh, sk] per head pair packed [128, 2048] via PE transposes
  - per head pair, per sq-block (512):
      STt = kT_h.T @ qT_h  -> PSUM [sk=128, sq=512] per sk-tile   (2-head
            row-group packed matmuls, K=64 each)
      causal mask: PSUM-accumulate identity.T @ maskneg tile (PE)
      p = exp(STt)  on ScalarE, [128, 1024] spans, PSUM->SBUF
      u'T [65, 512] += [v | 1].T-style matmul: lhsT = v' [sk,65], rhs = p
            (row 64 = softmax denominators, for free)
      u'T -> SBUF, reciprocal on row 64, PE-transpose back to [128, 65],
      o = u * recip (tensor_scalar), residual add (gpsimd), store.
"""

import numpy as np


def _import_concourse():
    try:
        import concourse.bacc  # noqa: F401
    except ImportError:
        import sys

        for p in ("/opt/trn_rl_repo", "/root/.axon_site/_ro/trn_rl_repo"):
            if p not in sys.path:
                sys.path.insert(0, p)
    import concourse.bacc as bacc
    import concourse.tile as tile
    from concourse import bass_utils, mybir

    return bacc, tile, bass_utils, mybir


def build_nc(S=2048, D=1024, HL=8, BLK=512):
    """Build the single-core Bass program (shared SPMD across the 8 cores)."""
    bacc, tile, bass_utils, mybir = _import_concourse()
    from contextlib import ExitStack

    dt = mybir.dt
    F32 = dt.float32
    F32R = dt.float32r
    BF16 = dt.bfloat16
    EXP = mybir.ActivationFunctionType.Exp

    DH = 64
    NST = S // 128  # seq 128-tiles
    ND = D // 128  # d_in 128-tiles
    NB = S // BLK  # sq blocks
    NSUB = BLK // 128  # 128-subtiles per sq block (4)
    NPAIR = HL // 2
    DQ = HL * DH  # local q width (512)
    SCALE = 1.0 / float(np.sqrt(D))

    nc = bacc.Bacc("TRN2", target_bir_lowering=False, debug=False)
    x_d = nc.dram_tensor("x", [S, D], F32, kind="ExternalInput").ap()
    xres_d = nc.dram_tensor("xres", [S, DQ], F32, kind="ExternalInput").ap()
    wq_d = nc.dram_tensor("wq", [DQ, D], F32, kind="ExternalInput").ap()
    k_d = nc.dram_tensor("k", [HL, S, DH], F32, kind="ExternalInput").ap()
    v_d = nc.dram_tensor("v", [HL, S, DH + 1], F32R, kind="ExternalInput").ap()
    id_d = nc.dram_tensor("ident", [128, 128], F32, kind="ExternalInput").ap()
    idr_d = nc.dram_tensor("identr", [128, 128], BF16, kind="ExternalInput").ap()
    mk_d = nc.dram_tensor("maskneg", [NSUB, 128, BLK], BF16, kind="ExternalInput").ap()
    out_d = nc.dram_tensor("out", [S, DQ], F32, kind="ExternalOutput").ap()

    with ExitStack() as ctx:
        tc = ctx.enter_context(tile.TileContext(nc))
        # SBUF pools
        const_p = ctx.enter_context(tc.tile_pool(name="const_p", bufs=1))
        xin_p = ctx.enter_context(tc.tile_pool(name="xin_p", bufs=3))
        xt_p = ctx.enter_context(tc.tile_pool(name="xt_p", bufs=1))
        wq_p = ctx.enter_context(tc.tile_pool(name="wq_p", bufs=2))
        wqt_p = ctx.enter_context(tc.tile_pool(name="wqt_p", bufs=1))
        qt_p = ctx.enter_context(tc.tile_pool(name="qt_p", bufs=2))
        kt_p = ctx.enter_context(tc.tile_pool(name="kt_p", bufs=2))
        kin_p = ctx.enter_context(tc.tile_pool(name="kin_p", bufs=2))
        v_p = ctx.enter_context(tc.tile_pool(name="v_p", bufs=3))
        exp_p = ctx.enter_context(tc.tile_pool(name="exp_p", bufs=4))
        ut_p = ctx.enter_context(tc.tile_pool(name="ut_p", bufs=3))
        o_p = ctx.enter_context(tc.tile_pool(name="o_p", bufs=2))
        xr_p = ctx.enter_context(tc.tile_pool(name="xr_p", bufs=2))
        # PSUM pools: st 2x[128,1024]=4 banks, u 2x bank = 2, mm 2x bank = 2
        pst = ctx.enter_context(tc.tile_pool(name="pst", bufs=2, space="PSUM"))
        pu = ctx.enter_context(tc.tile_pool(name="pu", bufs=2, space="PSUM"))
        pmm = ctx.enter_context(tc.tile_pool(name="pmm", bufs=2, space="PSUM"))

        # ---- constants ----
        id_sb = const_p.tile([128, 128], F32, name="id_sb")
        nc.sync.dma_start(out=id_sb, in_=id_d)
        id_r = const_p.tile([128, 128], BF16, name="id_r")
        nc.sync.dma_start(out=id_r, in_=idr_d)
        mk_r = const_p.tile([128, NSUB, BLK], BF16, name="mk_r")
        nc.sync.dma_start(out=mk_r, in_=mk_d.rearrange("r p f -> p r f"))

        # ---- k/v loads (pair 0 prefetched before the x/wq phases) ----
        kin_tiles = {}
        v_tiles = {}

        def load_kv(t):
            for h in (2 * t, 2 * t + 1):
                kin = kin_p.tile([128, NST, DH], F32, tag="kin", name=f"kin{h}")
                nc.gpsimd.dma_start(
                    out=kin, in_=k_d[h].rearrange("(t p) d -> p t d", p=128)
                )
                kin_tiles[h] = kin
                vt = v_p.tile([128, NST, DH + 1], F32R, tag="v", name=f"v{h}")
                nc.gpsimd.dma_start(
                    out=vt, in_=v_d[h].rearrange("(t p) d -> p t d", p=128)
                )
                v_tiles[h] = vt

        load_kv(0)

        # ---- Wq load + transpose -> wqT[c] = [128 (d_in), DQ (d_out)] ----
        wqt = []
        for c in range(ND):
            wt = wqt_p.tile([128, DQ], F32R, tag=f"wqt{c}", name=f"wqt{c}")
            wqt.append(wt)
        for m in range(DQ // 128):
            w = wq_p.tile([128, D], F32, tag="wq", name=f"wq_sb{m}")
            nc.sync.dma_start(out=w, in_=wq_d[m * 128 : (m + 1) * 128, :])
            for c in range(ND):
                ps = pmm.tile([128, 128], F32, tag="mm", name=f"wqtp{m}_{c}")
                nc.tensor.matmul(
                    ps, w[:, c * 128 : (c + 1) * 128], id_sb, is_transpose=True
                )
                nc.vector.tensor_copy(
                    out=wqt[c][:, m * 128 : (m + 1) * 128], in_=ps
                )

        # ---- x transpose -> SBUF-resident xT[c] = [128 (d_in), S] ----
        xt = []
        for c in range(ND):
            xtc = xt_p.tile([128, S], F32R, tag=f"xt{c}", name=f"xt{c}")
            xt.append(xtc)
        NSG = NST // 2
        for sg in range(NSG):
            xg = []
            for st in range(2 * sg, 2 * sg + 2):
                xi = xin_p.tile([128, D], F32, tag="xin", name=f"xin{st}")
                nc.sync.dma_start(out=xi, in_=x_d[st * 128 : (st + 1) * 128, :])
                xg.append(xi)
            for c in range(ND):
                ps = pmm.tile([128, 256], F32, tag="mm", name=f"xtp{sg}_{c}")
                for il in range(2):
                    nc.tensor.matmul(
                        ps[:, il * 128 : (il + 1) * 128],
                        xg[il][:, c * 128 : (c + 1) * 128],
                        id_sb,
                        is_transpose=True,
                    )
                nc.vector.tensor_copy(
                    out=xt[c][:, sg * 256 : (sg + 1) * 256], in_=ps
                )

        # ---- per head-pair ----
        for t in range(NPAIR):
            hA, hB = 2 * t, 2 * t + 1

            # kT pair tile [128, S]: rows 0:64 head A, 64:128 head B.
            # PSUM outputs must start at partition 0, so head B goes through
            # an SBUF staging tile and an SBUF->SBUF DMA partition shift.
            kt = kt_p.tile([128, S], BF16, tag="kt", name=f"kt{t}")
            kstg = kt_p.tile([64, S], BF16, tag="kstg", bufs=1, name=f"kstg{t}")
            for i0 in range(0, NST, 4):
                psA = pmm.tile([64, 512], F32, tag="mm", name=f"ktpa{t}_{i0}")
                psB = pmm.tile([64, 512], F32, tag="mm", name=f"ktpb{t}_{i0}")
                for il in range(4):
                    i = i0 + il
                    nc.tensor.matmul(
                        psA[:, il * 128 : (il + 1) * 128],
                        kin_tiles[hA][:, i, :],
                        id_sb,
                        is_transpose=True,
                    )
                    nc.tensor.matmul(
                        psB[:, il * 128 : (il + 1) * 128],
                        kin_tiles[hB][:, i, :],
                        id_sb,
                        is_transpose=True,
                    )
                nc.vector.tensor_copy(
                    out=kt[0:64, i0 * 128 : (i0 + 4) * 128], in_=psA
                )
                nc.vector.tensor_copy(
                    out=kstg[:, i0 * 128 : (i0 + 4) * 128], in_=psB
                )
            nc.gpsimd.dma_start(out=kt[64:128, :], in_=kstg)

            # qT tile for this pair: [128 (d_out slice), S].
            # Stream xT c-tiles back from DRAM; contract c outer so each
            # xT tile is loaded once per pair (NB psum banks would not fit,
            # so accumulate per nb in psum with c inner on a block of c).
            qt = qt_p.tile([128, S], BF16, tag="qt", name=f"qt{t}")
            for nbb in range(NB // 2):
                qpa = pmm.tile([128, BLK], F32, tag="mm", name=f"qpa{t}_{nbb}")
                qpb = pmm.tile([128, BLK], F32, tag="mm", name=f"qpb{t}_{nbb}")
                for c in range(ND):
                    wslice = wqt[c][:, t * 128 : (t + 1) * 128]
                    nc.tensor.matmul(
                        qpa, wslice,
                        xt[c][:, (2 * nbb) * BLK : (2 * nbb + 1) * BLK],
                        start=(c == 0), stop=(c == ND - 1),
                    )
                    nc.tensor.matmul(
                        qpb, wslice,
                        xt[c][:, (2 * nbb + 1) * BLK : (2 * nbb + 2) * BLK],
                        start=(c == 0), stop=(c == ND - 1),
                    )
                nc.vector.tensor_scalar_mul(
                    qt[:, (2 * nbb) * BLK : (2 * nbb + 1) * BLK], qpa, SCALE
                )
                nc.vector.tensor_scalar_mul(
                    qt[:, (2 * nbb + 1) * BLK : (2 * nbb + 2) * BLK], qpb, SCALE
                )

            # prefetch next pair's k/v while attention runs
            if t + 1 < NPAIR:
                load_kv(t + 1)

            vA, vB = v_tiles[hA], v_tiles[hB]

            # ---- attention for this pair ----
            for j in range(NB):
                ni = NSUB * (j + 1)  # valid sk tiles for this sq block
                jsl = slice(j * BLK, (j + 1) * BLK)
                uA = pu.tile([DH + 1, BLK], F32, tag="u", name=f"uA{t}_{j}")
                uB = pu.tile([DH + 1, BLK], F32, tag="u", name=f"uB{t}_{j}")
                for ih in range(0, ni, 2):
                    stA = pst.tile([128, 1024], F32, tag="st", name=f"stA{t}{j}{ih}")
                    stB = pst.tile([128, 1024], F32, tag="st", name=f"stB{t}{j}{ih}")
                    for il in range(2):
                        i = ih + il
                        sl = slice(il * BLK, (il + 1) * BLK)
                        isl = slice(i * 128, (i + 1) * 128)
                        r = i - NSUB * j
                        diag = r >= 0
                        nc.tensor.matmul(
                            stA[:, sl],
                            kt[0:64, isl],
                            qt[0:64, jsl],
                            start=True,
                            stop=not diag,
                            tile_position=(0, 0),
                        )
                        if diag:
                            nc.tensor.matmul(
                                stA[:, sl],
                                id_r,
                                mk_r[:, r, :],
                                start=False,
                                stop=True,
                                tile_position=(0, 0),
                            )
                        nc.tensor.matmul(
                            stB[:, sl],
                            kt[64:128, isl],
                            qt[64:128, jsl],
                            start=True,
                            stop=not diag,
                            tile_position=(64, 0),
                        )
                        if diag:
                            nc.tensor.matmul(
                                stB[:, sl],
                                id_r,
                                mk_r[:, r, :],
                                start=False,
                                stop=True,
                                tile_position=(0, 0),
                            )
                    eA = exp_p.tile([128, 1024], F32R, tag="exp", name=f"eA{t}{j}{ih}")
                    eB = exp_p.tile([128, 1024], F32R, tag="exp", name=f"eB{t}{j}{ih}")
                    nc.scalar.activation(out=eA, in_=stA, func=EXP)
                    nc.scalar.activation(out=eB, in_=stB, func=EXP)
                    for il in range(2):
                        i = ih + il
                        sl = slice(il * BLK, (il + 1) * BLK)
                        nc.tensor.matmul(
                            uA,
                            vA[:, i, :],
                            eA[:, sl],
                            start=(i == 0),
                            stop=(i == ni - 1),
                        )
                        nc.tensor.matmul(
                            uB,
                            vB[:, i, :],
                            eB[:, sl],
                            start=(i == 0),
                            stop=(i == ni - 1),
                        )

                # ---- normalize + output for this (pair, block) ----
                o4 = o_p.tile([128, NSUB, 128], F32, tag="o", name=f"o{t}{j}")
                xr = xr_p.tile([128, NSUB, 128], F32, tag="xr", name=f"xr{t}{j}")
                cols = slice(t * 128, (t + 1) * 128)
                nc.gpsimd.dma_start(
                    out=xr,
                    in_=xres_d[jsl, cols].rearrange("(s p) c -> p s c", p=128),
                )
                for hofs, uu in ((0, uA), (64, uB)):
                    ut = ut_p.tile([DH + 1, BLK], F32, tag="ut", name=f"ut{t}{j}{hofs}")
                    nc.vector.tensor_copy(out=ut, in_=uu)
                    for sub in range(NSUB):
                        un = pmm.tile([128, DH + 1], F32, tag="mm", name=f"un{t}{j}{hofs}{sub}")
                        nc.tensor.matmul(
                            un,
                            ut[:, sub * 128 : (sub + 1) * 128],
                            id_sb[0 : DH + 1, 0 : DH + 1],
                            is_transpose=True,
                        )
                        rc = ut_p.tile([128, 1], F32, tag="rc", name=f"rc{t}{j}{hofs}{sub}")
                        nc.vector.reciprocal(out=rc, in_=un[:, DH : DH + 1])
                        nc.vector.tensor_scalar_mul(
                            o4[:, sub, hofs : hofs + DH], un[:, 0:DH], rc
                        )
                nc.gpsimd.tensor_add(o4, o4, xr)
                nc.scalar.dma_start(
                    out=out_d[jsl, cols].rearrange("(s p) c -> p s c", p=128),
                    in_=o4,
                )

    nc.compile()
    return nc


def make_masks(BLK=512):
    NSUB = BLK // 128
    r = np.arange(NSUB)[:, None, None]
    p = np.arange(128)[None, :, None]
    f = np.arange(BLK)[None, None, :]
    return np.where(f >= p + 128 * r, 0.0, -30000.0).astype(np.float32)


def round_f32r(a):
    """Round fp32 values to fp32r (clear low 8 mantissa bits, RNE)."""
    u = np.ascontiguousarray(a, dtype=np.float32).view(np.uint32)
    frac = u & np.uint32(0xFF)
    up = (frac > 0x80) | ((frac == 0x80) & ((u >> np.uint32(8)) & np.uint32(1) == 1))
    u2 = (u & np.uint32(0xFFFFFF00)) + np.where(up, np.uint32(0x100), np.uint32(0))
    return u2.view(np.float32).reshape(a.shape)


def make_in_maps(x, k, v, Wq, HL=8):
    B, S, D = x.shape
    H = k.shape[1]
    G = H // HL
    DQ = HL * 64
    import ml_dtypes

    ident = np.eye(128, dtype=np.float32)
    identb = np.eye(128, dtype=ml_dtypes.bfloat16)
    maskneg = make_masks().astype(ml_dtypes.bfloat16)
    vones = np.concatenate(
        [round_f32r(v), np.ones(v.shape[:-1] + (1,), dtype=np.float32)], axis=-1
    )
    in_maps = []
    for core in range(B * G):
        b, g = divmod(core, G)
        in_maps.append(
            {
                "x": np.ascontiguousarray(x[b]),
                "xres": np.ascontiguousarray(x[b][:, g * DQ : (g + 1) * DQ]),
                "wq": np.ascontiguousarray(Wq[g * DQ : (g + 1) * DQ]),
                "k": np.ascontiguousarray(k[b, g * HL : (g + 1) * HL]),
                "v": vones[b, g * HL : (g + 1) * HL],
                "ident": ident,
                "identr": identb,
                "maskneg": maskneg,
            }
        )
    return in_maps


_NC_CACHE = {}


def _ensure_ntff_hook():
    """Provide antenv.axon_hooks if the image lacks it (trace=True path)."""
    import sys

    try:
        from antenv.axon_hooks import get_axon_ntff_profile_hook  # noqa: F401

        return
    except ImportError:
        pass
    import contextlib
    import ctypes
    import types

    so_path = "/opt/axon/libaxon_pjrt.so"
    hook = None
    try:
        lib = ctypes.CDLL(so_path)
        if hasattr(lib, "axon_start_nrt_profile"):
            lib.axon_start_nrt_profile.argtypes = [
                ctypes.POINTER(ctypes.c_int64),
                ctypes.c_size_t,
            ]
            lib.axon_start_nrt_profile.restype = ctypes.c_int64
            lib.axon_stop_nrt_profile.argtypes = [ctypes.c_char_p]
            lib.axon_stop_nrt_profile.restype = ctypes.c_int64

            @contextlib.contextmanager
            def _hook(output_dir, device_ids):
                import jax

                jax.devices()
                if device_ids:
                    ids = (ctypes.c_int64 * len(device_ids))(*device_ids)
                    rc = lib.axon_start_nrt_profile(ids, len(device_ids))
                else:
                    rc = lib.axon_start_nrt_profile(None, 0)
                if rc != 0:
                    raise RuntimeError(f"axon_start_nrt_profile rc={rc}")
                try:
                    yield
                finally:
                    n = lib.axon_stop_nrt_profile(str(output_dir).encode())
                    print(f"profile: {n} file(s) written to {output_dir}")

            hook = _hook
    except OSError:
        pass

    mod = types.ModuleType("antenv.axon_hooks")
    mod.get_axon_ntff_profile_hook = lambda: hook
    mod.set_axon_ntff_profile_hook = lambda h: None
    sys.modules["antenv.axon_hooks"] = mod


def run(x, k, v, Wq, trace=False, **kw):
    bacc, tile, bass_utils, mybir = _import_concourse()
    B, S, D = x.shape
    H = k.shape[1]
    HL = 8
    G = H // HL
    DQ = HL * 64
    if trace:
        _ensure_ntff_hook()
    key = (S, D, HL)
    if key not in _NC_CACHE:
        _NC_CACHE[key] = build_nc(S=S, D=D, HL=HL)
    nc = _NC_CACHE[key]
    in_maps = make_in_maps(x, k, v, Wq, HL=HL)
    res = bass_utils.run_bass_kernel_spmd(
        nc, in_maps, core_ids=list(range(B * G)), trace=trace, **kw
    )
    out = np.empty((B, S, D), dtype=x.dtype)
    for core in range(B * G):
        b, g = divmod(core, G)
        out[b][:, g * DQ : (g + 1) * DQ] = res.results[core]["out"]
    return out, res


def kernel(**inputs):
    out, _ = run(inputs["x"], inputs["k"], inputs["v"], inputs["Wq"])
    return out


# revision 30
# speedup vs baseline: 1.3733x; 1.0214x over previous
"""Trainium2 Bass kernel for nn_DecoderHead (B=4, S=2048, D=1024, H=16).

Sharding: 8 cores = 4 batches x 2 head-groups (8 heads each).
Per core: out[b, :, 512g:512(g+1)] = x[b, :, 512g:512(g+1)] + attn(heads 8g..8g+8).

Device-side design (per core, all fp32 / fp32r matmuls):
  - xT   [1024, 2048]  via PE transposes          (for the q projection)
  - WqT  [1024, 512]   via PE transposes
  - qT   [512, 2048]   = WqT.T @ xT tiles, scaled by 1/sqrt(D)    (PE, fp32r)
  - kT   [dh, sk] per head pair packed [128, 2048] via PE transposes
  - per head pair, per sq-block (512):
      STt = kT_h.T @ qT_h  -> PSUM [sk=128, sq=512] per sk-tile   (2-head
            row-group packed matmuls, K=64 each)
      causal mask: PSUM-accumulate identity.T @ maskneg tile (PE)
      p = exp(STt)  on ScalarE, [128, 1024] spans, PSUM->SBUF
      u'T [65, 512] += [v | 1].T-style matmul: lhsT = v' [sk,65], rhs = p
            (row 64 = softmax denominators, for free)
      u'T -> SBUF, reciprocal on row 64, PE-transpose back to [128, 65],
      o = u * recip (tensor_scalar), residual add (gpsimd), store.
"""

import numpy as np


def _import_concourse():
    try:
        import concourse.bacc  # noqa: F401
    except ImportError:
        import sys

        for p in ("/opt/trn_rl_repo", "/root/.axon_site/_ro/trn_rl_repo"):
            if p not in sys.path:
                sys.path.insert(0, p)
    import concourse.bacc as bacc
    import concourse.tile as tile
    from concourse import bass_utils, mybir

    return bacc, tile, bass_utils, mybir


def build_nc(S=2048, D=1024, HL=8, BLK=512):
    """Build the single-core Bass program (shared SPMD across the 8 cores)."""
    bacc, tile, bass_utils, mybir = _import_concourse()
    from contextlib import ExitStack

    dt = mybir.dt
    F32 = dt.float32
    F32R = dt.float32r
    BF16 = dt.bfloat16
    EXP = mybir.ActivationFunctionType.Exp

    DH = 64
    NST = S // 128  # seq 128-tiles
    ND = D // 128  # d_in 128-tiles
    NB = S // BLK  # sq blocks
    NSUB = BLK // 128  # 128-subtiles per sq block (4)
    NPAIR = HL // 2
    DQ = HL * DH  # local q width (512)
    SCALE = 1.0 / float(np.sqrt(D))

    nc = bacc.Bacc("TRN2", target_bir_lowering=False, debug=False)
    x_d = nc.dram_tensor("x", [S, D], F32, kind="ExternalInput").ap()
    xres_d = nc.dram_tensor("xres", [S, DQ], F32, kind="ExternalInput").ap()
    wq_d = nc.dram_tensor("wq", [DQ, D], F32, kind="ExternalInput").ap()
    k_d = nc.dram_tensor("k", [HL, S, DH], F32, kind="ExternalInput").ap()
    v_d = nc.dram_tensor("v", [HL, S, DH + 1], F32R, kind="ExternalInput").ap()
    id_d = nc.dram_tensor("ident", [128, 128], F32, kind="ExternalInput").ap()
    idr_d = nc.dram_tensor("identr", [128, 128], BF16, kind="ExternalInput").ap()
    mk_d = nc.dram_tensor("maskneg", [NSUB, 128, BLK], BF16, kind="ExternalInput").ap()
    out_d = nc.dram_tensor("out", [S, DQ], F32, kind="ExternalOutput").ap()

    with ExitStack() as ctx:
        tc = ctx.enter_context(tile.TileContext(nc))
        # SBUF pools
        const_p = ctx.enter_context(tc.tile_pool(name="const_p", bufs=1))
        xin_p = ctx.enter_context(tc.tile_pool(name="xin_p", bufs=5))
        xt_p = ctx.enter_context(tc.tile_pool(name="xt_p", bufs=1))
        wq_p = ctx.enter_context(tc.tile_pool(name="wq_p", bufs=2))
        wqt_p = ctx.enter_context(tc.tile_pool(name="wqt_p", bufs=1))
        qt_p = ctx.enter_context(tc.tile_pool(name="qt_p", bufs=2))
        kt_p = ctx.enter_context(tc.tile_pool(name="kt_p", bufs=2))
        kin_p = ctx.enter_context(tc.tile_pool(name="kin_p", bufs=2))
        v_p = ctx.enter_context(tc.tile_pool(name="v_p", bufs=3))
        exp_p = ctx.enter_context(tc.tile_pool(name="exp_p", bufs=4))
        ut_p = ctx.enter_context(tc.tile_pool(name="ut_p", bufs=3))
        o_p = ctx.enter_context(tc.tile_pool(name="o_p", bufs=2))
        xr_p = ctx.enter_context(tc.tile_pool(name="xr_p", bufs=2))
        # PSUM pools: st 2x[128,1024]=4 banks, u 2x bank = 2, mm 2x bank = 2
        pst = ctx.enter_context(tc.tile_pool(name="pst", bufs=2, space="PSUM"))
        pu = ctx.enter_context(tc.tile_pool(name="pu", bufs=2, space="PSUM"))
        pmm = ctx.enter_context(tc.tile_pool(name="pmm", bufs=2, space="PSUM"))

        # ---- constants ----
        id_sb = const_p.tile([128, 128], F32, name="id_sb")
        nc.sync.dma_start(out=id_sb, in_=id_d)
        id_r = const_p.tile([128, 128], BF16, name="id_r")
        nc.sync.dma_start(out=id_r, in_=idr_d)
        mk_r = const_p.tile([128, NSUB, BLK], BF16, name="mk_r")
        nc.sync.dma_start(out=mk_r, in_=mk_d.rearrange("r p f -> p r f"))

        # ---- k/v loads (pair 0 prefetched before the x/wq phases) ----
        kin_tiles = {}
        v_tiles = {}

        def load_kv(t):
            for h in (2 * t, 2 * t + 1):
                kin = kin_p.tile([128, NST, DH], F32, tag="kin", name=f"kin{h}")
                nc.gpsimd.dma_start(
                    out=kin, in_=k_d[h].rearrange("(t p) d -> p t d", p=128)
                )
                kin_tiles[h] = kin
                vt = v_p.tile([128, NST, DH + 1], F32R, tag="v", name=f"v{h}")
                nc.gpsimd.dma_start(
                    out=vt, in_=v_d[h].rearrange("(t p) d -> p t d", p=128)
                )
                v_tiles[h] = vt

        load_kv(0)

        # ---- x transpose -> SBUF-resident xT[c] = [128 (d_in), S] ----
        xt = []
        for c in range(ND):
            xtc = xt_p.tile([128, S], F32R, tag=f"xt{c}", name=f"xt{c}")
            xt.append(xtc)
        NSG = NST // 4
        for sg in range(NSG):
            xg = []
            for st in range(4 * sg, 4 * sg + 4):
                xi = xin_p.tile([128, D], F32, tag="xin", name=f"xin{st}")
                nc.sync.dma_start(out=xi, in_=x_d[st * 128 : (st + 1) * 128, :])
                xg.append(xi)
            for c in range(ND):
                ps = pmm.tile([128, 512], F32, tag="mm", name=f"xtp{sg}_{c}")
                for il in range(4):
                    nc.tensor.matmul(
                        ps[:, il * 128 : (il + 1) * 128],
                        xg[il][:, c * 128 : (c + 1) * 128],
                        id_sb,
                        is_transpose=True,
                    )
                nc.vector.tensor_copy(
                    out=xt[c][:, sg * 512 : (sg + 1) * 512], in_=ps
                )

        # ---- Wq load + transpose -> wqT[c] = [128 (d_in), DQ (d_out)] ----
        wqt = []
        for c in range(ND):
            wt = wqt_p.tile([128, DQ], F32R, tag=f"wqt{c}", name=f"wqt{c}")
            wqt.append(wt)
        for m in range(DQ // 128):
            w = wq_p.tile([128, D], F32, tag="wq", name=f"wq_sb{m}")
            nc.sync.dma_start(out=w, in_=wq_d[m * 128 : (m + 1) * 128, :])
            for c in range(ND):
                ps = pmm.tile([128, 128], F32, tag="mm", name=f"wqtp{m}_{c}")
                nc.tensor.matmul(
                    ps, w[:, c * 128 : (c + 1) * 128], id_sb, is_transpose=True
                )
                nc.vector.tensor_copy(
                    out=wqt[c][:, m * 128 : (m + 1) * 128], in_=ps
                )

        # ---- per head-pair ----
        for t in range(NPAIR):
            hA, hB = 2 * t, 2 * t + 1

            # kT pair tile [128, S]: rows 0:64 head A, 64:128 head B.
            # PSUM outputs must start at partition 0, so head B goes through
            # an SBUF staging tile and an SBUF->SBUF DMA partition shift.
            kt = kt_p.tile([128, S], BF16, tag="kt", name=f"kt{t}")
            kstg = kt_p.tile([64, S], BF16, tag="kstg", bufs=1, name=f"kstg{t}")
            for i0 in range(0, NST, 4):
                psA = pmm.tile([64, 512], F32, tag="mm", name=f"ktpa{t}_{i0}")
                psB = pmm.tile([64, 512], F32, tag="mm", name=f"ktpb{t}_{i0}")
                for il in range(4):
                    i = i0 + il
                    nc.tensor.matmul(
                        psA[:, il * 128 : (il + 1) * 128],
                        kin_tiles[hA][:, i, :],
                        id_sb,
                        is_transpose=True,
                    )
                    nc.tensor.matmul(
                        psB[:, il * 128 : (il + 1) * 128],
                        kin_tiles[hB][:, i, :],
                        id_sb,
                        is_transpose=True,
                    )
                nc.vector.tensor_copy(
                    out=kt[0:64, i0 * 128 : (i0 + 4) * 128], in_=psA
                )
                nc.vector.tensor_copy(
                    out=kstg[:, i0 * 128 : (i0 + 4) * 128], in_=psB
                )
            nc.gpsimd.dma_start(out=kt[64:128, :], in_=kstg)

            # qT tile for this pair: [128 (d_out slice), S].
            # Stream xT c-tiles back from DRAM; contract c outer so each
            # xT tile is loaded once per pair (NB psum banks would not fit,
            # so accumulate per nb in psum with c inner on a block of c).
            qt = qt_p.tile([128, S], BF16, tag="qt", name=f"qt{t}")
            for nbb in range(NB // 2):
                qpa = pmm.tile([128, BLK], F32, tag="mm", name=f"qpa{t}_{nbb}")
                qpb = pmm.tile([128, BLK], F32, tag="mm", name=f"qpb{t}_{nbb}")
                for c in range(ND):
                    wslice = wqt[c][:, t * 128 : (t + 1) * 128]
                    nc.tensor.matmul(
                        qpa, wslice,
                        xt[c][:, (2 * nbb) * BLK : (2 * nbb + 1) * BLK],
                        start=(c == 0), stop=(c == ND - 1),
                    )
                    nc.tensor.matmul(
                        qpb, wslice,
                        xt[c][:, (2 * nbb + 1) * BLK : (2 * nbb + 2) * BLK],
                        start=(c == 0), stop=(c == ND - 1),
                    )
                nc.vector.tensor_scalar_mul(
                    qt[:, (2 * nbb) * BLK : (2 * nbb + 1) * BLK], qpa, SCALE
                )
                nc.vector.tensor_scalar_mul(
                    qt[:, (2 * nbb + 1) * BLK : (2 * nbb + 2) * BLK], qpb, SCALE
                )

            # prefetch next pair's k/v while attention runs
            if t + 1 < NPAIR:
                load_kv(t + 1)

            vA, vB = v_tiles[hA], v_tiles[hB]

            # ---- attention for this pair ----
            for j in range(NB):
                ni = NSUB * (j + 1)  # valid sk tiles for this sq block
                jsl = slice(j * BLK, (j + 1) * BLK)
                uA = pu.tile([DH + 1, BLK], F32, tag="u", name=f"uA{t}_{j}")
                uB = pu.tile([DH + 1, BLK], F32, tag="u", name=f"uB{t}_{j}")
                for ih in range(0, ni, 2):
                    stA = pst.tile([128, 1024], F32, tag="st", name=f"stA{t}{j}{ih}")
                    stB = pst.tile([128, 1024], F32, tag="st", name=f"stB{t}{j}{ih}")
                    for il in range(2):
                        i = ih + il
                        sl = slice(il * BLK, (il + 1) * BLK)
                        isl = slice(i * 128, (i + 1) * 128)
                        r = i - NSUB * j
                        diag = r >= 0
                        nc.tensor.matmul(
                            stA[:, sl],
                            kt[0:64, isl],
                            qt[0:64, jsl],
                            start=True,
                            stop=not diag,
                            tile_position=(0, 0),
                        )
                        if diag:
                            nc.tensor.matmul(
                                stA[:, sl],
                                id_r,
                                mk_r[:, r, :],
                                start=False,
                                stop=True,
                                tile_position=(0, 0),
                            )
                        nc.tensor.matmul(
                            stB[:, sl],
                            kt[64:128, isl],
                            qt[64:128, jsl],
                            start=True,
                            stop=not diag,
                            tile_position=(64, 0),
                        )
                        if diag:
                            nc.tensor.matmul(
                                stB[:, sl],
                                id_r,
                                mk_r[:, r, :],
                                start=False,
                                stop=True,
                                tile_position=(0, 0),
                            )
                    eA = exp_p.tile([128, 1024], F32R, tag="exp", name=f"eA{t}{j}{ih}")
                    eB = exp_p.tile([128, 1024], F32R, tag="exp", name=f"eB{t}{j}{ih}")
                    nc.scalar.activation(out=eA, in_=stA, func=EXP)
                    nc.scalar.activation(out=eB, in_=stB, func=EXP)
                    for il in range(2):
                        i = ih + il
                        sl = slice(il * BLK, (il + 1) * BLK)
                        nc.tensor.matmul(
                            uA,
                            vA[:, i, :],
                            eA[:, sl],
                            start=(i == 0),
                            stop=(i == ni - 1),
                        )
                        nc.tensor.matmul(
                            uB,
                            vB[:, i, :],
                            eB[:, sl],
                            start=(i == 0),
                            stop=(i == ni - 1),
                        )

                # ---- normalize + output for this (pair, block) ----
                o4 = o_p.tile([128, NSUB, 128], F32, tag="o", name=f"o{t}{j}")
                xr = xr_p.tile([128, NSUB, 128], F32, tag="xr", name=f"xr{t}{j}")
                cols = slice(t * 128, (t + 1) * 128)
                nc.gpsimd.dma_start(
                    out=xr,
                    in_=xres_d[jsl, cols].rearrange("(s p) c -> p s c", p=128),
                )
                for hofs, uu in ((0, uA), (64, uB)):
                    ut = ut_p.tile([DH + 1, BLK], F32, tag="ut", name=f"ut{t}{j}{hofs}")
                    nc.vector.tensor_copy(out=ut, in_=uu)
                    for sub in range(NSUB):
                        un = pmm.tile([128, DH + 1], F32, tag="mm", name=f"un{t}{j}{hofs}{sub}")
                        nc.tensor.matmul(
                            un,
                            ut[:, sub * 128 : (sub + 1) * 128],
                            id_sb[0 : DH + 1, 0 : DH + 1],
                            is_transpose=True,
                        )
                        rc = ut_p.tile([128, 1], F32, tag="rc", name=f"rc{t}{j}{hofs}{sub}")
                        nc.vector.reciprocal(out=rc, in_=un[:, DH : DH + 1])
                        nc.vector.tensor_scalar_mul(
                            o4[:, sub, hofs : hofs + DH], un[:, 0:DH], rc
                        )
                nc.gpsimd.tensor_add(o4, o4, xr)
                nc.scalar.dma_start(
                    out=out_d[jsl, cols].rearrange("(s p) c -> p s c", p=128),
                    in_=o4,
                )

    nc.compile()
    return nc


def make_masks(BLK=512):
    NSUB = BLK // 128
    r = np.arange(NSUB)[:, None, None]
    p = np.arange(128)[None, :, None]
    f = np.arange(BLK)[None, None, :]
    return np.where(f >= p + 128 * r, 0.0, -30000.0).astype(np.float32)


def round_f32r(a):
    """Round fp32 values to fp32r (clear low 8 mantissa bits, RNE)."""
    u = np.ascontiguousarray(a, dtype=np.float32).view(np.uint32)
    frac = u & np.uint32(0xFF)
    up = (frac > 0x80) | ((frac == 0x80) & ((u >> np.uint32(8)) & np.uint32(1) == 1))
    u2 = (u & np.uint32(0xFFFFFF00)) + np.where(up, np.uint32(0x100), np.uint32(0))
    return u2.view(np.float32).reshape(a.shape)


def make_in_maps(x, k, v, Wq, HL=8):
    B, S, D = x.shape
    H = k.shape[1]
    G = H // HL
    DQ = HL * 64
    import ml_dtypes

    ident = np.eye(128, dtype=np.float32)
    identb = np.eye(128, dtype=ml_dtypes.bfloat16)
    maskneg = make_masks().astype(ml_dtypes.bfloat16)
    vones = np.concatenate(
        [round_f32r(v), np.ones(v.shape[:-1] + (1,), dtype=np.float32)], axis=-1
    )
    in_maps = []
    for core in range(B * G):
        b, g = divmod(core, G)
        in_maps.append(
            {
                "x": np.ascontiguousarray(x[b]),
                "xres": np.ascontiguousarray(x[b][:, g * DQ : (g + 1) * DQ]),
                "wq": np.ascontiguousarray(Wq[g * DQ : (g + 1) * DQ]),
                "k": np.ascontiguousarray(k[b, g * HL : (g + 1) * HL]),
                "v": vones[b, g * HL : (g + 1) * HL],
                "ident": ident,
                "identr": identb,
                "maskneg": maskneg,
            }
        )
    return in_maps


_NC_CACHE = {}


def _ensure_ntff_hook():
    """Provide antenv.axon_hooks if the image lacks it (trace=True path)."""
    import sys

    try:
        from antenv.axon_hooks import get_axon_ntff_profile_hook  # noqa: F401

        return
    except ImportError:
        pass
    import contextlib
    import ctypes
    import types

    so_path = "/opt/axon/libaxon_pjrt.so"
    hook = None
    try:
        lib = ctypes.CDLL(so_path)
        if hasattr(lib, "axon_start_nrt_profile"):
            lib.axon_start_nrt_profile.argtypes = [
                ctypes.POINTER(ctypes.c_int64),
                ctypes.c_size_t,
            ]
            lib.axon_start_nrt_profile.restype = ctypes.c_int64
            lib.axon_stop_nrt_profile.argtypes = [ctypes.c_char_p]
            lib.axon_stop_nrt_profile.restype = ctypes.c_int64

            @contextlib.contextmanager
            def _hook(output_dir, device_ids):
                import jax

                jax.devices()
                if device_ids:
                    ids = (ctypes.c_int64 * len(device_ids))(*device_ids)
                    rc = lib.axon_start_nrt_profile(ids, len(device_ids))
                else:
                    rc = lib.axon_start_nrt_profile(None, 0)
                if rc != 0:
                    raise RuntimeError(f"axon_start_nrt_profile rc={rc}")
                try:
                    yield
                finally:
                    n = lib.axon_stop_nrt_profile(str(output_dir).encode())
                    print(f"profile: {n} file(s) written to {output_dir}")

            hook = _hook
    except OSError:
        pass

    mod = types.ModuleType("antenv.axon_hooks")
    mod.get_axon_ntff_profile_hook = lambda: hook
    mod.set_axon_ntff_profile_hook = lambda h: None
    sys.modules["antenv.axon_hooks"] = mod


def run(x, k, v, Wq, trace=False, **kw):
    bacc, tile, bass_utils, mybir = _import_concourse()
    B, S, D = x.shape
    H = k.shape[1]
    HL = 8
    G = H // HL
    DQ = HL * 64
    if trace:
        _ensure_ntff_hook()
    key = (S, D, HL)
    if key not in _NC_CACHE:
        _NC_CACHE[key] = build_nc(S=S, D=D, HL=HL)
    nc = _NC_CACHE[key]
    in_maps = make_in_maps(x, k, v, Wq, HL=HL)
    res = bass_utils.run_bass_kernel_spmd(
        nc, in_maps, core_ids=list(range(B * G)), trace=trace, **kw
    )
    out = np.empty((B, S, D), dtype=x.dtype)
    for core in range(B * G):
        b, g = divmod(core, G)
        out[b][:, g * DQ : (g + 1) * DQ] = res.results[core]["out"]
    return out, res


def kernel(**inputs):
    out, _ = run(inputs["x"], inputs["k"], inputs["v"], inputs["Wq"])
    return out
